# revision 1
# baseline (speedup 1.0000x reference)
"""LoRA MultiheadAttention on 8 NeuronCores (Bass/Tile).

Sharding: 32 (batch, head) attention slices -> 4 heads x 1 batch per core.
Cores 0-3 take batch 0, cores 4-7 batch 1; core c handles heads
(c%4)*4 .. (c%4)*4+3, i.e. a contiguous 256-wide slice of the head dims.

Per-core math (all big matmuls bf16 on PE, fp32 PSUM accumulate):
  xaT   (1152, 2048) = [X^T; ones-row; zero pad]  (bias via ones row)
  qkT   = wqk^T-slices @ X  -> Q^T, K^T in (head-dim, T) layout
          (q pre-scaled by 1/sqrt(hd); LoRA K accumulated into same PSUM)
  V     = X @ Wv-slice (natural (T, dv) layout, per-head 65-wide blocks with
          a ones column -> PV matmul emits the softmax denominator for free)
  S^T   = K^T.T-slices @ Q^T  (tj on partitions, ti free)  [K=64 contraction]
  P^T   = exp(S^T)  on ACT  (no max-subtraction: |scores| <~ 3 by construction)
  O^T   = V_aug.T @ P^T  accumulated over tj; row 64 = denom
  norm  : denom row broadcast across 64 partitions via K=1 PE matmul with a
          ones column, reciprocal on DVE, multiply -> normalized O^T (bf16)
  out   = O^T.T @ out_w-slice^T  (T, 1024) fp32 partial, summed on host.

b_v is folded into the V matmul ones-row bias; out_b added on host.
"""

import sys

sys.path.insert(0, "/opt/trn_rl_repo")

import math
from contextlib import ExitStack

import ml_dtypes
import numpy as np

import concourse.bass as bass
import concourse.tile as tile
from concourse import bacc
from concourse import mybir
from concourse.bass_utils import run_bass_kernel_spmd

BF16 = ml_dtypes.bfloat16
F32 = mybir.dt.float32
BF = mybir.dt.bfloat16

T = 2048
D = 1024
H = 16
HD = 64
R = 16
BSZ = 2
SCALE = 16.0
NCORES = 8
HPC = 4  # heads per core
CD = HPC * HD  # 256 head dims per core
VW = HD + 1  # V block width per head (ones column appended)
KPAD = 1152  # 1024 X rows + 1 ones row, padded to 9 k-tiles of 128
NKT = KPAD // 128
P = 128
NTT = T // P  # 16 row tiles
HF = T // 2  # 1024: ti processed in two halves


def build_nc():
    nc = bass.Bass()
    xa = nc.dram_tensor("xa", [KPAD, T], BF, kind="ExternalInput")
    wqk = nc.dram_tensor("wqk", [KPAD, 2 * CD], BF, kind="ExternalInput")
    wv = nc.dram_tensor("wv", [KPAD, HPC * VW], BF, kind="ExternalInput")
    ab = nc.dram_tensor("ab", [KPAD, 3 * R], BF, kind="ExternalInput")
    kbm = nc.dram_tensor("kbm", [R, CD], BF, kind="ExternalInput")
    vbm = nc.dram_tensor("vbm", [R, HPC * VW], BF, kind="ExternalInput")
    wo = nc.dram_tensor("wo", [CD, D], BF, kind="ExternalInput")
    out = nc.dram_tensor("out", [T, D], F32, kind="ExternalOutput")

    with tile.TileContext(nc) as tc, ExitStack() as ctx:
        singles = ctx.enter_context(tc.tile_pool(name="singles", bufs=1))

        xa_t = [singles.tile([P, T], BF, name=f"xa{i}", tag=f"xa{i}") for i in range(NKT)]
        wqk_t = [singles.tile([P, 2 * CD], BF, name=f"wqk{i}", tag=f"wqk{i}") for i in range(NKT)]
        wv_t = [singles.tile([P, HPC * VW], BF, name=f"wv{i}", tag=f"wv{i}") for i in range(NKT)]
        ab_t = [singles.tile([P, 3 * R], BF, name=f"ab{i}", tag=f"ab{i}") for i in range(NKT)]
        kb_t = singles.tile([R, CD], BF, tag="kb")
        vb_t = singles.tile([R, HPC * VW], BF, tag="vb")
        wo_t = [singles.tile([P, D], BF, name=f"wo{i}", tag=f"wo{i}") for i in range(2)]
        for i in range(NKT):
            nc.sync.dma_start(out=xa_t[i], in_=xa[i * P : (i + 1) * P, :])
            nc.sync.dma_start(out=wqk_t[i], in_=wqk[i * P : (i + 1) * P, :])
            nc.sync.dma_start(out=wv_t[i], in_=wv[i * P : (i + 1) * P, :])
            nc.sync.dma_start(out=ab_t[i], in_=ab[i * P : (i + 1) * P, :])
        nc.sync.dma_start(out=kb_t, in_=kbm[:, :])
        nc.sync.dma_start(out=vb_t, in_=vbm[:, :])
        for i in range(2):
            nc.sync.dma_start(out=wo_t[i], in_=wo[i * P : (i + 1) * P, :])

        ones_t = singles.tile([1, HD], F32, tag="ones")
        nc.vector.memset(ones_t, 1.0)

        qk_sb = [singles.tile([P, T], BF, name=f"qk{i}", tag=f"qk{i}") for i in range(4)]
        ak_sb = singles.tile([R, T], BF, tag="ak")
        av_sb = singles.tile([R, T], BF, tag="av")
        v_sb = [singles.tile([P, HPC * VW], BF, name=f"v{i}", tag=f"v{i}") for i in range(NTT)]
        oT_sb = [singles.tile([P, T], BF, name=f"oT{i}", tag=f"oT{i}") for i in range(2)]

        # Phase A: A_kv^T = [k_a; v_a] @ X   (32, T)
        with tc.tile_pool(name="pA", bufs=2, space="PSUM") as pA:
            for ch in range(4):
                cs = slice(ch * 512, (ch + 1) * 512)
                pa = pA.tile([3 * R, 512], F32, tag="pa")
                for kt in range(8):  # ab rows >= 1024 are zero; skip 9th tile
                    nc.tensor.matmul(
                        pa,
                        lhsT=ab_t[kt],
                        rhs=xa_t[kt][:, cs],
                        start=(kt == 0),
                        stop=(kt == 7),
                    )
                nc.vector.tensor_copy(ak_sb[:, cs], pa[0:R, :])
                nc.vector.tensor_copy(av_sb[:, cs], pa[2 * R : 3 * R, :])

        # Phase B: Q^T, K^T (4 m-tiles of 128) with LoRA-K accumulated
        with tc.tile_pool(name="pB", bufs=3, space="PSUM") as pB:
            for m in range(4):
                for ch in range(4):
                    cs = slice(ch * 512, (ch + 1) * 512)
                    pq = pB.tile([P, 512], F32, tag="pq")
                    for kt in range(NKT):
                        nc.tensor.matmul(
                            pq,
                            lhsT=wqk_t[kt][:, m * P : (m + 1) * P],
                            rhs=xa_t[kt][:, cs],
                            start=(kt == 0),
                            stop=(kt == NKT - 1 and m < 2),
                        )
                    if m >= 2:
                        nc.tensor.matmul(
                            pq,
                            lhsT=kb_t[:, (m - 2) * P : (m - 1) * P],
                            rhs=ak_sb[:, cs],
                            start=False,
                            stop=True,
                        )
                    nc.vector.tensor_copy(qk_sb[m][:, cs], pq)

        # Phase C: V natural (T, 4*65) with ones cols + b_v via ones-row, LoRA-V
        with tc.tile_pool(name="pC", bufs=3, space="PSUM") as pC:
            for mt in range(NTT):
                ms = slice(mt * P, (mt + 1) * P)
                pv = pC.tile([P, HPC * VW], F32, tag="pv")
                for kt in range(NKT):
                    nc.tensor.matmul(
                        pv,
                        lhsT=xa_t[kt][:, ms],
                        rhs=wv_t[kt],
                        start=(kt == 0),
                        stop=False,
                    )
                nc.tensor.matmul(
                    pv, lhsT=av_sb[:, ms], rhs=vb_t, start=False, stop=True
                )
                nc.vector.tensor_copy(v_sb[mt], pv)

        # Phase D+E: attention units (half-outer, head-inner), software-
        # pipelined normalize (unit i's normalize emitted after unit i+1's
        # matmuls so PE never stalls at unit boundaries), denominator
        # broadcast via DRAM round-trip DMA (stride-0 partition read) instead
        # of a PE matmul, and half-0 out-proj overlapped with half-1 attention.
        with (
            tc.tile_pool(name="pS", bufs=3, space="PSUM") as pS,
            tc.tile_pool(name="pO", bufs=2, space="PSUM") as pO,
            tc.tile_pool(name="pE", bufs=1, space="PSUM") as pE,
            tc.tile_pool(name="pP", bufs=6) as pP,
            tc.tile_pool(name="pN", bufs=2) as pN,
            tc.tile_pool(name="pD", bufs=2, space="DRAM") as pD,
            tc.tile_pool(name="pOut", bufs=3) as pOut,
        ):
            def emit_unit(half, h):
                qT = qk_sb[h // 2][(h % 2) * HD : (h % 2) * HD + HD, :]
                kT = qk_sb[2 + h // 2][(h % 2) * HD : (h % 2) * HD + HD, :]
                po = pO.tile([VW, HF], F32, tag="po", name=f"po_{half}_{h}")
                pts = {}

                def emit_pv(tjp):
                    for q2 in range(2):
                        nc.tensor.matmul(
                            po[:, q2 * 512 : (q2 + 1) * 512],
                            lhsT=v_sb[tjp][:, h * VW : (h + 1) * VW],
                            rhs=pts.pop((tjp, q2)),
                            start=(tjp == 0),
                            stop=(tjp == NTT - 1),
                        )

                # PV shifted one tj behind S so exp(tj) overlaps S(tj+1) and
                # PE never waits on ACT (keeps the >=3us continuous-execution
                # window that promotes PE to the full 2.4 GHz p-state).
                for tj in range(NTT):
                    for q2 in range(2):
                        qs = slice(half * HF + q2 * 512, half * HF + (q2 + 1) * 512)
                        ps = pS.tile([P, 512], F32, tag="spsum", name=f"ps_{half}_{h}_{tj}_{q2}")
                        nc.tensor.matmul(
                            ps,
                            lhsT=kT[:, tj * P : (tj + 1) * P],
                            rhs=qT[:, qs],
                            start=True,
                            stop=True,
                        )
                        pt = pP.tile([P, 512], BF, tag="pt", name=f"pt_{half}_{h}_{tj}_{q2}")
                        nc.scalar.activation(pt, ps, mybir.ActivationFunctionType.Exp)
                        pts[(tj, q2)] = pt
                    if tj > 0:
                        emit_pv(tj - 1)
                emit_pv(NTT - 1)
                return po

            def emit_norm(half, h, po):
                hs = slice(half * HF, (half + 1) * HF)
                den = pN.tile([1, HF], F32, tag="den", name=f"den_{half}_{h}")
                nc.vector.tensor_copy(den, po[HD:VW, :])
                dr = pD.tile([1, HF], F32, tag="dr", name=f"dr_{half}_{h}")
                nc.sync.dma_start(out=dr, in_=den)
                den64 = pN.tile([HD, HF], F32, tag="den64", name=f"den64_{half}_{h}")
                nc.sync.dma_start(
                    out=den64,
                    in_=bass.AP(tensor=dr.tensor, offset=dr.offset, ap=[[0, HD], [1, HF]]),
                )
                rec = pN.tile([HD, HF], F32, tag="rec", name=f"rec_{half}_{h}")
                nc.vector.reciprocal(rec, den64)
                nc.vector.tensor_mul(
                    oT_sb[h // 2][(h % 2) * HD : (h % 2) * HD + HD, hs],
                    po[0:HD, :],
                    rec,
                )

            def emit_outproj(half):
                for mt in range(half * 8, (half + 1) * 8):
                    ms = slice(mt * P, (mt + 1) * P)
                    ob = pOut.tile([P, D], F32, tag="ob", name=f"ob_{mt}")
                    for ch in range(2):
                        cs = slice(ch * 512, (ch + 1) * 512)
                        po2 = pE.tile([P, 512], F32, tag="po2", name=f"po2_{mt}_{ch}")
                        for kt2 in range(2):
                            nc.tensor.matmul(
                                po2,
                                lhsT=oT_sb[kt2][:, ms],
                                rhs=wo_t[kt2][:, cs],
                                start=(kt2 == 0),
                                stop=(kt2 == 1),
                            )
                        nc.vector.tensor_copy(ob[:, cs], po2)
                    nc.sync.dma_start(out=out[ms, :], in_=ob)

            units = [(half, h) for half in range(2) for h in range(HPC)]
            prev = None
            for i, (half, h) in enumerate(units):
                po = emit_unit(half, h)
                if prev is not None:
                    emit_norm(prev[0], prev[1], prev[2])
                    if i == 4:
                        emit_outproj(0)
                prev = (half, h, po)
            emit_norm(prev[0], prev[1], prev[2])
            emit_outproj(1)

    # bass.Bass's finalize skips Bacc's wait-splitting passes; walrus allows
    # at most 1 sync wait per instruction (2 for event semaphores), so run
    # just those two passes here.
    import bass_rust as _bass_rust

    _bass_rust.move_matmul_waits_to_ldweights(nc.m)
    _bass_rust.generate_event_semaphores(nc)
    return nc


def prepare_in_maps(inputs):
    q = np.asarray(inputs["query"], np.float32)
    ipw = np.asarray(inputs["in_proj_weight"], np.float32)
    ipb = np.asarray(inputs["in_proj_bias"], np.float32)
    out_w = np.asarray(inputs["out_w"], np.float32)
    k_a = np.asarray(inputs["k_a"], np.float32)
    k_b = np.asarray(inputs["k_b"], np.float32)
    v_a = np.asarray(inputs["v_a"], np.float32)
    v_b = np.asarray(inputs["v_b"], np.float32)
    qscale = 1.0 / math.sqrt(HD)
    sl = SCALE / R

    in_maps = []
    for c in range(NCORES):
        bb = c // 4
        s = (c % 4) * CD
        e = s + CD
        X = q[:, bb, :]

        xa = np.zeros((KPAD, T), np.float32)
        xa[:D] = X.T
        xa[D] = 1.0

        wqk = np.zeros((KPAD, 2 * CD), np.float32)
        wqk[:D, :CD] = ipw[s:e].T * qscale
        wqk[D, :CD] = ipb[s:e] * qscale
        wqk[:D, CD:] = ipw[D + s : D + e].T
        wqk[D, CD:] = ipb[D + s : D + e]

        wv = np.zeros((KPAD, HPC * VW), np.float32)
        for j in range(HPC):
            wv[:D, j * VW : j * VW + HD] = ipw[2 * D + s + j * HD : 2 * D + s + (j + 1) * HD].T
            wv[D, j * VW : j * VW + HD] = ipb[2 * D + s + j * HD : 2 * D + s + (j + 1) * HD]
            wv[D, j * VW + HD] = 1.0

        ab = np.zeros((KPAD, 3 * R), np.float32)
        ab[:D, :R] = k_a.T
        ab[:D, 2 * R :] = v_a.T

        kbm = k_b[:, s:e] * sl

        vbm = np.zeros((R, HPC * VW), np.float32)
        for j in range(HPC):
            vbm[:, j * VW : j * VW + HD] = v_b[:, s + j * HD : s + (j + 1) * HD] * sl

        wo = out_w[:, s:e].T

        in_maps.append(
            {
                "xa": xa.astype(BF16),
                "wqk": wqk.astype(BF16),
                "wv": wv.astype(BF16),
                "ab": ab.astype(BF16),
                "kbm": kbm.astype(BF16),
                "vbm": vbm.astype(BF16),
                "wo": wo.astype(BF16),
            }
        )
    return in_maps


def assemble_output(inputs, results):
    out_b = np.asarray(inputs["out_b"], np.float32)
    out = np.zeros((T, BSZ, D), np.float32)
    for c in range(NCORES):
        out[:, c // 4, :] += results[c]["out"]
    out += out_b[None, None, :]
    return out


def kernel(**inputs):
    nc = build_nc()
    in_maps = prepare_in_maps(inputs)
    res = run_bass_kernel_spmd(nc, in_maps, core_ids=list(range(NCORES)))
    return assemble_output(inputs, res.results)



# revision 3
# speedup vs baseline: 1.4433x; 1.4433x over previous
"""LoRA MultiheadAttention on 8 NeuronCores (Bass/Tile), v2.

Sharding: 32 (batch, head) attention slices -> 4 heads x 1 batch per core.
Cores 0-3 take batch 0, cores 4-7 batch 1; core c handles heads
(c%4)*4 .. (c%4)*4+3, i.e. a contiguous 256-wide slice of the head dims.

Per-core math (all big matmuls bf16 on PE, fp32 PSUM accumulate):
  xaT   (1152, 2048) = [X^T; ones-row; zero pad]  (bias via ones row)
  qkT   = wqk^T-slices @ X  -> Q^T, K^T in (head-dim, T) layout
          (q pre-scaled by 1/sqrt(hd); LoRA K accumulated into same PSUM)
  V     = X @ Wv-slice (natural (T, dv) layout, per-head 65-wide blocks with
          a ones column -> PV matmul emits the softmax denominator for free)

Attention is processed as head PAIRS so the K=64 score matmuls run two-at-
a-time on disjoint PE row groups (row-tiling: head-even weights on array
rows 0-63, head-odd on 64-127) -> ~2x PE throughput on S^T.

  S^T   = K^T.T-slices @ Q^T  into [128, 1024] PSUM tiles (2 banks)
  P^T   : exp split across TWO engines running concurrently:
          - ACT: real exp, (1024+352)/1.2 ns per tile
          - DVE: one-op Schraudolph: pt_bits_i16 = rint(s*128/ln2 + B),
            bitcast to bf16 (|rel err| <~ 3%, mean-zero calibrated; only a
            minority of tiles, softmax renormalizes so output err ~0.5%)
  O^T   = V_aug.T @ P^T accumulated over tj; row 64 = denominator
  norm  : po evacuated to SBUF bf16 (frees PSUM early); denominator row
          round-trips through DRAM reshaped to [128, 8] so the DVE
          reciprocal uses 128 lanes (0.13us vs 6.5us for the broadcast
          [64,1024] reciprocal), then a stride-0 DMA broadcasts 1/den and
          one bf16 2x-mode multiply normalizes into oT_sb
  out   = O^T.T @ out_w-slice^T  (T, 1024) fp32 partial, summed on host;
          PSUM->SBUF copies alternate ACT/DVE engines.

b_v is folded into the V matmul ones-row bias; out_b added on host.
"""

import sys

sys.path.insert(0, "/opt/trn_rl_repo")

import math
from contextlib import ExitStack

import ml_dtypes
import numpy as np

import concourse.bass as bass
import concourse.tile as tile
from concourse import mybir
from concourse.alu_op_type import AluOpType
from concourse.bass_utils import run_bass_kernel_spmd

BF16 = ml_dtypes.bfloat16
F32 = mybir.dt.float32
BF = mybir.dt.bfloat16
I16 = mybir.dt.int16

T = 2048
D = 1024
H = 16
HD = 64
R = 16
BSZ = 2
SCALE = 16.0
NCORES = 8
HPC = 4  # heads per core
CD = HPC * HD  # 256 head dims per core
VW = HD + 1  # V block width per head (ones column appended)
KPAD = 1152  # 1024 X rows + 1 ones row, padded to 9 k-tiles of 128
NKT = KPAD // 128
P = 128
NTT = T // P  # 16 row tiles
HF = T // 2  # 1024: ti processed in two halves

# Schraudolph-bf16 exp constants: i16 = rint(x * 128/ln2 + (127*128 - C)),
# bitcast to bf16. C calibrated for ~zero mean multiplicative error on the
# observed score distribution (|s| < 4).
EXP_A = 128.0 / math.log(2.0)
EXP_B = 127.0 * 128.0 - 7.3
# tj tiles of the odd head of each pair that go to DVE (rest go to ACT).
DVE_TJ = frozenset(range(16)) - {5, 10, 15}


def build_nc():
    nc = bass.Bass()
    xa = nc.dram_tensor("xa", [KPAD, T], BF, kind="ExternalInput")
    wqk = nc.dram_tensor("wqk", [KPAD, 2 * CD], BF, kind="ExternalInput")
    wv = nc.dram_tensor("wv", [KPAD, HPC * VW], BF, kind="ExternalInput")
    ab = nc.dram_tensor("ab", [KPAD, 3 * R], BF, kind="ExternalInput")
    kbm = nc.dram_tensor("kbm", [R, CD], BF, kind="ExternalInput")
    vbm = nc.dram_tensor("vbm", [R, HPC * VW], BF, kind="ExternalInput")
    wo = nc.dram_tensor("wo", [CD, D], BF, kind="ExternalInput")
    out = nc.dram_tensor("out", [T, D], F32, kind="ExternalOutput")

    with tile.TileContext(nc) as tc, ExitStack() as ctx:
        singles = ctx.enter_context(tc.tile_pool(name="singles", bufs=1))

        xa_t = [singles.tile([P, T], BF, name=f"xa{i}", tag=f"xa{i}") for i in range(NKT)]
        wqk_t = [singles.tile([P, 2 * CD], BF, name=f"wqk{i}", tag=f"wqk{i}") for i in range(NKT)]
        wv_t = [singles.tile([P, HPC * VW], BF, name=f"wv{i}", tag=f"wv{i}") for i in range(NKT)]
        ab_t = [singles.tile([P, 3 * R], BF, name=f"ab{i}", tag=f"ab{i}") for i in range(NKT)]
        kb_t = singles.tile([R, CD], BF, tag="kb")
        vb_t = singles.tile([R, HPC * VW], BF, tag="vb")
        wo_t = [singles.tile([P, D], BF, name=f"wo{i}", tag=f"wo{i}") for i in range(2)]
        for i in range(NKT):
            nc.sync.dma_start(out=xa_t[i], in_=xa[i * P : (i + 1) * P, :])
            nc.sync.dma_start(out=wqk_t[i], in_=wqk[i * P : (i + 1) * P, :])
            nc.sync.dma_start(out=wv_t[i], in_=wv[i * P : (i + 1) * P, :])
            nc.sync.dma_start(out=ab_t[i], in_=ab[i * P : (i + 1) * P, :])
        nc.sync.dma_start(out=kb_t, in_=kbm[:, :])
        nc.sync.dma_start(out=vb_t, in_=vbm[:, :])
        for i in range(2):
            nc.sync.dma_start(out=wo_t[i], in_=wo[i * P : (i + 1) * P, :])

        qk_sb = [singles.tile([P, T], BF, name=f"qk{i}", tag=f"qk{i}") for i in range(4)]
        ak_sb = singles.tile([R, T], BF, tag="ak")
        av_sb = singles.tile([R, T], BF, tag="av")
        v_sb = [singles.tile([P, HPC * VW], BF, name=f"v{i}", tag=f"v{i}") for i in range(NTT)]
        oT_sb = [singles.tile([P, T], BF, name=f"oT{i}", tag=f"oT{i}") for i in range(2)]

        # Phase A: A_kv^T = [k_a; v_a] @ X   (32, T)
        with tc.tile_pool(name="pA", bufs=2, space="PSUM") as pA:
            for ch in range(4):
                cs = slice(ch * 512, (ch + 1) * 512)
                pa = pA.tile([3 * R, 512], F32, tag="pa")
                for kt in range(8):  # ab rows >= 1024 are zero; skip 9th tile
                    nc.tensor.matmul(
                        pa,
                        lhsT=ab_t[kt],
                        rhs=xa_t[kt][:, cs],
                        start=(kt == 0),
                        stop=(kt == 7),
                    )
                nc.vector.tensor_copy(ak_sb[:, cs], pa[0:R, :])
                nc.vector.tensor_copy(av_sb[:, cs], pa[2 * R : 3 * R, :])

        # Phase B: Q^T, K^T (4 m-tiles of 128) with LoRA-K accumulated
        with tc.tile_pool(name="pB", bufs=3, space="PSUM") as pB:
            for m in range(4):
                for ch in range(4):
                    cs = slice(ch * 512, (ch + 1) * 512)
                    pq = pB.tile([P, 512], F32, tag="pq")
                    for kt in range(NKT):
                        nc.tensor.matmul(
                            pq,
                            lhsT=wqk_t[kt][:, m * P : (m + 1) * P],
                            rhs=xa_t[kt][:, cs],
                            start=(kt == 0),
                            stop=(kt == NKT - 1 and m < 2),
                        )
                    if m >= 2:
                        nc.tensor.matmul(
                            pq,
                            lhsT=kb_t[:, (m - 2) * P : (m - 1) * P],
                            rhs=ak_sb[:, cs],
                            start=False,
                            stop=True,
                        )
                    nc.vector.tensor_copy(qk_sb[m][:, cs], pq)

        # Phase C: V natural (T, 4*65) with ones cols + b_v via ones-row, LoRA-V
        with tc.tile_pool(name="pC", bufs=3, space="PSUM") as pC:
            for mt in range(NTT):
                ms = slice(mt * P, (mt + 1) * P)
                pv = pC.tile([P, HPC * VW], F32, tag="pv")
                for kt in range(NKT):
                    nc.tensor.matmul(
                        pv,
                        lhsT=xa_t[kt][:, ms],
                        rhs=wv_t[kt],
                        start=(kt == 0),
                        stop=False,
                    )
                nc.tensor.matmul(
                    pv, lhsT=av_sb[:, ms], rhs=vb_t, start=False, stop=True
                )
                nc.vector.tensor_copy(v_sb[mt], pv)

        # Phase D+E: attention in head pairs. Per (half, hp) the two heads'
        # S matmuls run concurrently on disjoint PE row groups; exp is split
        # ACT/DVE; PV accumulates into per-head [65, 1024] PSUM; the
        # denominator path reshapes through DRAM for a 128-lane reciprocal.
        with (
            tc.tile_pool(name="pS", bufs=2, space="PSUM") as pS,
            tc.tile_pool(name="pO", bufs=2, space="PSUM") as pO,
            tc.tile_pool(name="pP", bufs=6) as pP,
            tc.tile_pool(name="pEv", bufs=3) as pEv,
            tc.tile_pool(name="pN", bufs=3) as pN,
            tc.tile_pool(name="pD", bufs=3, space="DRAM") as pD,
        ):
            def emit_pair(half, hp):
                hs = slice(half * HF, (half + 1) * HF)
                po = [
                    pO.tile([VW, HF], F32, tag="po", name=f"po_{half}_{hp}_{h2}")
                    for h2 in range(2)
                ]
                pts = {}

                def emit_pv(t):
                    for h2 in range(2):
                        hh = 2 * hp + h2
                        pt = pts.pop((t, h2))
                        for c in range(2):
                            nc.tensor.matmul(
                                po[h2][:, c * 512 : (c + 1) * 512],
                                lhsT=v_sb[t][:, hh * VW : (hh + 1) * VW],
                                rhs=pt[:, c * 512 : (c + 1) * 512],
                                start=(t == 0),
                                stop=(t == NTT - 1),
                            )

                for tj in range(NTT):
                    ps = [
                        pS.tile([P, HF], F32, tag="s", name=f"ps_{half}_{hp}_{tj}_{h2}")
                        for h2 in range(2)
                    ]
                    # interleave chunks so the two heads' K=64 matmuls run
                    # concurrently on PE row groups 0-63 / 64-127
                    for c in range(2):
                        for h2 in range(2):
                            rs = slice(h2 * HD, (h2 + 1) * HD)
                            nc.tensor.matmul(
                                ps[h2][:, c * 512 : (c + 1) * 512],
                                lhsT=qk_sb[2 + hp][rs, tj * P : (tj + 1) * P],
                                rhs=qk_sb[hp][rs, half * HF + c * 512 : half * HF + (c + 1) * 512],
                                start=True,
                                stop=True,
                            )
                    for h2 in range(2):
                        pt = pP.tile([P, HF], BF, tag="pt", name=f"pt_{half}_{hp}_{tj}_{h2}")
                        if h2 == 1 and tj in DVE_TJ:
                            nc.vector.tensor_scalar(
                                pt.bitcast(I16), ps[h2], EXP_A, EXP_B,
                                AluOpType.mult, AluOpType.add,
                            )
                        else:
                            nc.scalar.activation(
                                pt, ps[h2], mybir.ActivationFunctionType.Exp
                            )
                        pts[(tj, h2)] = pt
                    if tj > 0:
                        emit_pv(tj - 1)
                emit_pv(NTT - 1)
                return po

            def emit_norm(half, hp, po):
                hs = slice(half * HF, (half + 1) * HF)
                for h2 in range(2):
                    # evacuate the whole [65, HF] PSUM tile (incl. denom row)
                    # to bf16 SBUF so the PSUM banks free up early
                    ev = pEv.tile([VW, HF], BF, tag="ev", name=f"ev_{half}_{hp}_{h2}")
                    nc.vector.tensor_copy(ev, po[h2])
                    dr = pD.tile([1, HF], BF, tag="dr", name=f"dr_{half}_{hp}_{h2}")
                    nc.sync.dma_start(out=dr, in_=ev[HD:VW, :])
                    den128 = pN.tile([P, HF // P], BF, tag="d128", name=f"d128_{half}_{hp}_{h2}")
                    nc.sync.dma_start(
                        out=den128,
                        in_=bass.AP(tensor=dr.tensor, offset=dr.offset,
                                    ap=[[HF // P, P], [1, HF // P]]),
                    )
                    rec = pN.tile([P, HF // P], BF, tag="rec", name=f"rec_{half}_{hp}_{h2}")
                    with nc.allow_low_precision(
                        reason="softmax denom ~2048; bf16 recip adds ~0.4% row scale noise, within tolerance"
                    ):
                        nc.vector.reciprocal(rec, den128)
                    rr = pD.tile([1, HF], BF, tag="rr", name=f"rr_{half}_{hp}_{h2}")
                    nc.sync.dma_start(
                        out=bass.AP(tensor=rr.tensor, offset=rr.offset,
                                    ap=[[HF // P, P], [1, HF // P]]),
                        in_=rec,
                    )
                    rb = pN.tile([HD, HF], BF, tag="rb", name=f"rb_{half}_{hp}_{h2}")
                    nc.sync.dma_start(
                        out=rb,
                        in_=bass.AP(tensor=rr.tensor, offset=rr.offset,
                                    ap=[[0, HD], [1, HF]]),
                    )
                    nc.vector.tensor_mul(
                        oT_sb[hp][h2 * HD : (h2 + 1) * HD, hs],
                        ev[0:HD, :],
                        rb,
                    )

            prev = None
            for half in range(2):
                for hp in range(2):
                    po = emit_pair(half, hp)
                    if prev is not None:
                        emit_norm(*prev)
                    prev = (half, hp, po)
            emit_norm(*prev)

        # out-projection: (T, 1024) fp32 partial = O^T.T @ wo, PSUM->SBUF
        # copies alternate DVE/ACT so neither engine is the tail bottleneck
        with (
            tc.tile_pool(name="pE", bufs=2, space="PSUM") as pE,
            tc.tile_pool(name="pOut", bufs=3) as pOut,
        ):
            for mt in range(NTT):
                ms = slice(mt * P, (mt + 1) * P)
                ob = pOut.tile([P, D], F32, tag="ob", name=f"ob_{mt}")
                for ch in range(2):
                    cs = slice(ch * 512, (ch + 1) * 512)
                    po2 = pE.tile([P, 512], F32, tag="po2", name=f"po2_{mt}_{ch}")
                    for kt2 in range(2):
                        nc.tensor.matmul(
                            po2,
                            lhsT=oT_sb[kt2][:, ms],
                            rhs=wo_t[kt2][:, cs],
                            start=(kt2 == 0),
                            stop=(kt2 == 1),
                        )
                    if (mt * 2 + ch) % 2 == 0:
                        nc.vector.tensor_copy(ob[:, cs], po2)
                    else:
                        nc.scalar.copy(ob[:, cs], po2)
                nc.sync.dma_start(out=out[ms, :], in_=ob)

    # bass.Bass's finalize skips Bacc's wait-splitting passes; walrus allows
    # at most 1 sync wait per instruction (2 for event semaphores), so run
    # just those two passes here.
    import bass_rust as _bass_rust

    _bass_rust.move_matmul_waits_to_ldweights(nc.m)
    _bass_rust.generate_event_semaphores(nc)
    return nc


def prepare_in_maps(inputs):
    q = np.asarray(inputs["query"], np.float32)
    ipw = np.asarray(inputs["in_proj_weight"], np.float32)
    ipb = np.asarray(inputs["in_proj_bias"], np.float32)
    out_w = np.asarray(inputs["out_w"], np.float32)
    k_a = np.asarray(inputs["k_a"], np.float32)
    k_b = np.asarray(inputs["k_b"], np.float32)
    v_a = np.asarray(inputs["v_a"], np.float32)
    v_b = np.asarray(inputs["v_b"], np.float32)
    qscale = 1.0 / math.sqrt(HD)
    sl = SCALE / R

    in_maps = []
    for c in range(NCORES):
        bb = c // 4
        s = (c % 4) * CD
        e = s + CD
        X = q[:, bb, :]

        xa = np.zeros((KPAD, T), np.float32)
        xa[:D] = X.T
        xa[D] = 1.0

        wqk = np.zeros((KPAD, 2 * CD), np.float32)
        wqk[:D, :CD] = ipw[s:e].T * qscale
        wqk[D, :CD] = ipb[s:e] * qscale
        wqk[:D, CD:] = ipw[D + s : D + e].T
        wqk[D, CD:] = ipb[D + s : D + e]

        wv = np.zeros((KPAD, HPC * VW), np.float32)
        for j in range(HPC):
            wv[:D, j * VW : j * VW + HD] = ipw[2 * D + s + j * HD : 2 * D + s + (j + 1) * HD].T
            wv[D, j * VW : j * VW + HD] = ipb[2 * D + s + j * HD : 2 * D + s + (j + 1) * HD]
            wv[D, j * VW + HD] = 1.0

        ab = np.zeros((KPAD, 3 * R), np.float32)
        ab[:D, :R] = k_a.T
        ab[:D, 2 * R :] = v_a.T

        kbm = k_b[:, s:e] * sl

        vbm = np.zeros((R, HPC * VW), np.float32)
        for j in range(HPC):
            vbm[:, j * VW : j * VW + HD] = v_b[:, s + j * HD : s + (j + 1) * HD] * sl

        wo = out_w[:, s:e].T

        in_maps.append(
            {
                "xa": xa.astype(BF16),
                "wqk": wqk.astype(BF16),
                "wv": wv.astype(BF16),
                "ab": ab.astype(BF16),
                "kbm": kbm.astype(BF16),
                "vbm": vbm.astype(BF16),
                "wo": wo.astype(BF16),
            }
        )
    return in_maps


def assemble_output(inputs, results):
    out_b = np.asarray(inputs["out_b"], np.float32)
    out = np.zeros((T, BSZ, D), np.float32)
    for c in range(NCORES):
        out[:, c // 4, :] += results[c]["out"]
    out += out_b[None, None, :]
    return out


def kernel(**inputs):
    nc = build_nc()
    in_maps = prepare_in_maps(inputs)
    res = run_bass_kernel_spmd(nc, in_maps, core_ids=list(range(NCORES)))
    return assemble_output(inputs, res.results)


# revision 11
# speedup vs baseline: 1.4816x; 1.0265x over previous
"""LoRA MultiheadAttention on 8 NeuronCores (Bass/Tile), v3.

Sharding: 32 (batch, head) attention slices -> 4 heads x 1 batch per core.
Cores 0-3 take batch 0, cores 4-7 batch 1; core c handles heads
(c%4)*4 .. (c%4)*4+3, i.e. a contiguous 256-wide slice of the head dims.

The PE is drain-bound on TRN2: every matmul costs N columns of fp32 PSUM
drain at 1 col/cycle regardless of K or M, so the kernel is organized to
keep the PE instruction stream dense end-to-end:

  prologue: A^T LoRA activations, Q^T/K^T for heads 0-1 (B m-tiles 0,2),
            V row-tiles 0-2 -- just enough to start attention.
  attention (4 head-pairs x 16 tj iterations): S^T -> exp -> PV, with the
            REMAINING phase-B m-tiles, phase-C row-tiles and the half-0
            out-projection interleaved into the PE stream as filler (2 MMs
            per tj iteration from a chain queue) so exp waits never idle
            the PE; all filler accumulates in a single spare PSUM bank.
  exp split across ACT (real exp) and DVE (one-op Schraudolph bf16:
            i16 = rint(s*128/ln2 + B) bitcast bf16, mean-zero calibrated,
            ~41% of tiles) so neither engine gates the PE.
  norm:     po evacuated to bf16 SBUF immediately (frees PSUM banks);
            denominator row round-trips through DRAM reshaped to [128, 8]
            so the reciprocal uses 128 DVE lanes; stride-0 DMA broadcast,
            one 2x-mode bf16 multiply into oT_sb.
  epilogue: half-1 out-projection, PSUM->SBUF copies alternating ACT/DVE.

b_v is folded into the V matmul ones-row bias; out_b added on host.
"""

import sys

sys.path.insert(0, "/opt/trn_rl_repo")

import math
from contextlib import ExitStack

import ml_dtypes
import numpy as np

import concourse.bass as bass
import concourse.tile as tile
from concourse import mybir
from concourse.alu_op_type import AluOpType
from concourse.bass_utils import run_bass_kernel_spmd

BF16 = ml_dtypes.bfloat16
F32 = mybir.dt.float32
BF = mybir.dt.bfloat16
I16 = mybir.dt.int16

T = 2048
D = 1024
H = 16
HD = 64
R = 16
BSZ = 2
SCALE = 16.0
NCORES = 8
HPC = 4  # heads per core
CD = HPC * HD  # 256 head dims per core
VW = HD + 1  # V block width per head (ones column appended)
KPAD = 1152  # 1024 X rows + 1 ones row, padded to 9 k-tiles of 128
NKT = KPAD // 128
P = 128
NTT = T // P  # 16 row tiles
HF = T // 2  # 1024: ti processed in two halves

# Schraudolph-bf16 exp: i16 = rint(x * 128/ln2 + (127*128 - C)), bitcast bf16.
EXP_A = 128.0 / math.log(2.0)
EXP_B = 127.0 * 128.0 - 7.3
# tj tiles of the odd head of each pair whose exp goes to DVE (rest ACT).
DVE_TJ = frozenset(range(16)) - {5, 10, 15}
N_FILL = 2  # filler PE instructions drained per tj iteration


def build_nc():
    nc = bass.Bass()
    xa = nc.dram_tensor("xa", [KPAD, T], BF, kind="ExternalInput")
    wqk = nc.dram_tensor("wqk", [KPAD, 2 * CD], BF, kind="ExternalInput")
    wv = nc.dram_tensor("wv", [KPAD, HPC * VW], BF, kind="ExternalInput")
    ab = nc.dram_tensor("ab", [KPAD, 3 * R], BF, kind="ExternalInput")
    kbm = nc.dram_tensor("kbm", [R, CD], BF, kind="ExternalInput")
    vbm = nc.dram_tensor("vbm", [R, HPC * VW], BF, kind="ExternalInput")
    wo = nc.dram_tensor("wo", [CD, D], BF, kind="ExternalInput")
    out = nc.dram_tensor("out", [T, D], F32, kind="ExternalOutput")

    with tile.TileContext(nc) as tc, ExitStack() as ctx:
        singles = ctx.enter_context(tc.tile_pool(name="singles", bufs=1))

        xa_t = [singles.tile([P, T], BF, name=f"xa{i}", tag=f"xa{i}") for i in range(NKT)]
        wqk_t = [singles.tile([P, 2 * CD], BF, name=f"wqk{i}", tag=f"wqk{i}") for i in range(NKT)]
        wv_t = [singles.tile([P, HPC * VW], BF, name=f"wv{i}", tag=f"wv{i}") for i in range(NKT)]
        ab_t = [singles.tile([P, 3 * R], BF, name=f"ab{i}", tag=f"ab{i}") for i in range(NKT)]
        kb_t = singles.tile([R, CD], BF, tag="kb")
        vb_t = singles.tile([R, HPC * VW], BF, tag="vb")
        wo_t = [singles.tile([P, D], BF, name=f"wo{i}", tag=f"wo{i}") for i in range(2)]
        for i in range(NKT):
            nc.sync.dma_start(out=xa_t[i], in_=xa[i * P : (i + 1) * P, :])
            nc.sync.dma_start(out=wqk_t[i], in_=wqk[i * P : (i + 1) * P, :])
            nc.sync.dma_start(out=wv_t[i], in_=wv[i * P : (i + 1) * P, :])
            nc.sync.dma_start(out=ab_t[i], in_=ab[i * P : (i + 1) * P, :])
        nc.sync.dma_start(out=kb_t, in_=kbm[:, :])
        nc.sync.dma_start(out=vb_t, in_=vbm[:, :])
        for i in range(2):
            nc.sync.dma_start(out=wo_t[i], in_=wo[i * P : (i + 1) * P, :])

        qk_sb = [singles.tile([P, T], BF, name=f"qk{i}", tag=f"qk{i}") for i in range(4)]
        ak_sb = singles.tile([R, T], BF, tag="ak")
        av_sb = singles.tile([R, T], BF, tag="av")
        v_sb = [singles.tile([P, HPC * VW], BF, name=f"v{i}", tag=f"v{i}") for i in range(NTT)]
        oT_sb = [singles.tile([P, T], BF, name=f"oT{i}", tag=f"oT{i}") for i in range(2)]
        pOut = ctx.enter_context(tc.tile_pool(name="pOut", bufs=3))
        ob_sb = {}

        # ---- emission helpers for B (Q^T/K^T) and C (V) chunks ----------
        def emit_b_chunk(pool, m, ch, copy_engine):
            """Full accumulation chain for one (m-tile, 512-col chunk) of
            Q^T/K^T: 9 (+1 LoRA) matmuls into one PSUM bank, then copy."""
            cs = slice(ch * 512, (ch + 1) * 512)
            pq = pool.tile([P, 512], F32, tag="aux", name=f"pq_{m}_{ch}")
            steps = []
            for kt in range(NKT):
                steps.append(
                    lambda kt=kt, pq=pq: nc.tensor.matmul(
                        pq,
                        lhsT=wqk_t[kt][:, m * P : (m + 1) * P],
                        rhs=xa_t[kt][:, cs],
                        start=(kt == 0),
                        stop=(kt == NKT - 1 and m < 2),
                    )
                )
            if m >= 2:
                steps.append(
                    lambda pq=pq: nc.tensor.matmul(
                        pq,
                        lhsT=kb_t[:, (m - 2) * P : (m - 1) * P],
                        rhs=ak_sb[:, cs],
                        start=False,
                        stop=True,
                    )
                )
            steps.append(
                lambda pq=pq: copy_engine(qk_sb[m][:, cs], pq)
            )
            return steps

        def emit_c_chunk(pool, mt, copy_engine):
            ms = slice(mt * P, (mt + 1) * P)
            pv = pool.tile([P, HPC * VW], F32, tag="aux", name=f"pv_{mt}")
            steps = []
            for kt in range(NKT):
                steps.append(
                    lambda kt=kt, pv=pv: nc.tensor.matmul(
                        pv,
                        lhsT=xa_t[kt][:, ms],
                        rhs=wv_t[kt],
                        start=(kt == 0),
                        stop=False,
                    )
                )
            steps.append(
                lambda pv=pv: nc.tensor.matmul(
                    pv, lhsT=av_sb[:, ms], rhs=vb_t, start=False, stop=True
                )
            )
            steps.append(lambda pv=pv: copy_engine(v_sb[mt], pv))
            return steps

        def emit_outproj_chunk(pool, mt, ch, copy_engine):
            ms = slice(mt * P, (mt + 1) * P)
            cs = slice(ch * 512, (ch + 1) * 512)
            po2 = pool.tile([P, 512], F32, tag="aux", name=f"po2_{mt}_{ch}")
            steps = []
            if ch == 0:
                def mkob(mt=mt):
                    ob_sb[mt] = pOut.tile([P, D], F32, tag="ob", name=f"ob_{mt}")
                steps.append(mkob)
            for kt2 in range(2):
                steps.append(
                    lambda kt2=kt2, po2=po2: nc.tensor.matmul(
                        po2,
                        lhsT=oT_sb[kt2][:, ms],
                        rhs=wo_t[kt2][:, cs],
                        start=(kt2 == 0),
                        stop=(kt2 == 1),
                    )
                )

            def fin(po2=po2, mt=mt, ch=ch):
                copy_engine(ob_sb[mt][:, cs], po2)
                if ch == 1:
                    nc.sync.dma_start(out=out[ms, :], in_=ob_sb[mt])

            steps.append(fin)
            return steps

        # Phase A: A_kv^T = [k_a; v_a] @ X   (32, T)
        with tc.tile_pool(name="pA", bufs=2, space="PSUM") as pA:
            for ch in range(4):
                cs = slice(ch * 512, (ch + 1) * 512)
                pa = pA.tile([3 * R, 512], F32, tag="pa")
                for kt in range(8):  # ab rows >= 1024 are zero; skip 9th tile
                    nc.tensor.matmul(
                        pa,
                        lhsT=ab_t[kt],
                        rhs=xa_t[kt][:, cs],
                        start=(kt == 0),
                        stop=(kt == 7),
                    )
                nc.vector.tensor_copy(ak_sb[:, cs], pa[0:R, :])
                nc.vector.tensor_copy(av_sb[:, cs], pa[2 * R : 3 * R, :])

        # Prologue: all of Q^T/K^T (phase B) plus V row-tiles 0-2; the rest
        # of phase C is paced into pair (0,0)'s PE stream as filler.
        with tc.tile_pool(name="pPro", bufs=3, space="PSUM") as pPro:
            for m in (0, 2, 1, 3):
                for ch in range(4):
                    for step in emit_b_chunk(pPro, m, ch, nc.vector.tensor_copy):
                        step()
            for mt in range(3):
                for step in emit_c_chunk(pPro, mt, nc.vector.tensor_copy):
                    step()

        # Attention + interleaved filler
        with (
            tc.tile_pool(name="pS", bufs=3, space="PSUM") as pS,
            tc.tile_pool(name="pO", bufs=2, space="PSUM") as pO,
            tc.tile_pool(name="pX", bufs=1, space="PSUM") as pX,
            tc.tile_pool(name="pP", bufs=8) as pP,
            tc.tile_pool(name="pEv", bufs=3) as pEv,
            tc.tile_pool(name="pN", bufs=3) as pN,
            tc.tile_pool(name="pD", bufs=3, space="DRAM") as pD,
        ):
            filler = []  # flat list of zero-arg emit closures (PE/copy steps)

            def alt_copy(i=[0]):
                # alternate PSUM->SBUF copies between DVE and ACT
                i[0] += 1
                return nc.vector.tensor_copy if i[0] % 2 else nc.scalar.copy

            for mt in range(3, NTT):
                filler.extend(emit_c_chunk(pX, mt, alt_copy()))
            fill_pos = [0]

            def drain_filler(n):
                q = filler
                i = fill_pos[0]
                for _ in range(n):
                    if i >= len(q):
                        break
                    q[i]()
                    i += 1
                fill_pos[0] = i

            def emit_pair(half, hp, fill_rate):
                po = [
                    pO.tile([VW, HF], F32, tag="po", name=f"po_{half}_{hp}_{h2}")
                    for h2 in range(2)
                ]
                pts = {}

                def emit_pv(t):
                    for h2 in range(2):
                        hh = 2 * hp + h2
                        pt = pts.pop((t, h2))
                        for c in range(2):
                            nc.tensor.matmul(
                                po[h2][:, c * 512 : (c + 1) * 512],
                                lhsT=v_sb[t][:, hh * VW : (hh + 1) * VW],
                                rhs=pt[:, c * 512 : (c + 1) * 512],
                                start=(t == 0),
                                stop=(t == NTT - 1),
                            )

                for tj in range(NTT):
                    pt = [
                        pP.tile([P, HF], BF, tag="pt", name=f"pt_{half}_{hp}_{tj}_{h2}")
                        for h2 in range(2)
                    ]

                    def emit_exp(h2, c, s_psum):
                        ptc = pt[h2][:, c * 512 : (c + 1) * 512]
                        if h2 == 1 and tj in DVE_TJ:
                            nc.vector.tensor_scalar(
                                ptc.bitcast(I16), s_psum, EXP_A, EXP_B,
                                AluOpType.mult, AluOpType.add,
                            )
                        else:
                            nc.scalar.activation(
                                ptc, s_psum, mybir.ActivationFunctionType.Exp
                            )

                    # S matmuls chunk by chunk, exp emitted right behind each
                    # pair of chunks so pS slots recycle quickly
                    for c in range(2):
                        pcs = []
                        for h2 in range(2):
                            s_ps = pS.tile(
                                [P, 512], F32, tag="s",
                                name=f"ps_{half}_{hp}_{tj}_{h2}_{c}",
                            )
                            rs = slice(h2 * HD, (h2 + 1) * HD)
                            nc.tensor.matmul(
                                s_ps,
                                lhsT=qk_sb[2 + hp][rs, tj * P : (tj + 1) * P],
                                rhs=qk_sb[hp][rs, half * HF + c * 512 : half * HF + (c + 1) * 512],
                                start=True,
                                stop=True,
                            )
                            pcs.append(s_ps)
                        for h2 in range(2):
                            emit_exp(h2, c, pcs[h2])
                    pts[(tj, 0)] = pt[0]
                    pts[(tj, 1)] = pt[1]
                    drain_filler(fill_rate)
                    if tj > 0:
                        emit_pv(tj - 1)
                emit_pv(NTT - 1)
                return po

            def emit_norm(half, hp, po):
                hs = slice(half * HF, (half + 1) * HF)
                for h2 in range(2):
                    ev = pEv.tile([VW, HF], BF, tag="ev", name=f"ev_{half}_{hp}_{h2}")
                    nc.vector.tensor_copy(ev, po[h2])
                    dr = pD.tile([1, HF], BF, tag="dr", name=f"dr_{half}_{hp}_{h2}")
                    nc.sync.dma_start(out=dr, in_=ev[HD:VW, :])
                    den128 = pN.tile([P, HF // P], BF, tag="d128", name=f"d128_{half}_{hp}_{h2}")
                    nc.sync.dma_start(
                        out=den128,
                        in_=bass.AP(tensor=dr.tensor, offset=dr.offset,
                                    ap=[[HF // P, P], [1, HF // P]]),
                    )
                    rec = pN.tile([P, HF // P], BF, tag="rec", name=f"rec_{half}_{hp}_{h2}")
                    with nc.allow_low_precision(
                        reason="softmax denom ~2048; bf16 recip adds ~0.4% row scale noise"
                    ):
                        nc.vector.reciprocal(rec, den128)
                    rr = pD.tile([1, HF], BF, tag="rr", name=f"rr_{half}_{hp}_{h2}")
                    nc.sync.dma_start(
                        out=bass.AP(tensor=rr.tensor, offset=rr.offset,
                                    ap=[[HF // P, P], [1, HF // P]]),
                        in_=rec,
                    )
                    rb = pN.tile([HD, HF], BF, tag="rb", name=f"rb_{half}_{hp}_{h2}")
                    nc.sync.dma_start(
                        out=rb,
                        in_=bass.AP(tensor=rr.tensor, offset=rr.offset,
                                    ap=[[0, HD], [1, HF]]),
                    )
                    nc.vector.tensor_mul(
                        oT_sb[hp][h2 * HD : (h2 + 1) * HD, hs],
                        ev[0:HD, :],
                        rb,
                    )

            prev = None
            for half in range(2):
                for hp in range(2):
                    po = emit_pair(half, hp, fill_rate={(0, 0): 12, (1, 1): 5}.get((half, hp), 2))
                    if prev is not None:
                        emit_norm(*prev)
                        if (half, hp) == (1, 0):
                            # half-0 oT complete: queue its out-projection
                            for mt in range(NTT // 2):
                                for ch in range(2):
                                    filler.extend(
                                        emit_outproj_chunk(pX, mt, ch, alt_copy())
                                    )
                    prev = (half, hp, po)
            emit_norm(*prev)
            drain_filler(len(filler))  # flush any remaining filler

        # Epilogue: half-1 out-projection
        with tc.tile_pool(name="pE", bufs=2, space="PSUM") as pE:
            eng = [nc.vector.tensor_copy, nc.scalar.copy]
            for mt in range(NTT // 2, NTT):
                for ch in range(2):
                    for step in emit_outproj_chunk(pE, mt, ch, eng[(mt + ch) % 2]):
                        step()

    import bass_rust as _bass_rust

    _bass_rust.move_matmul_waits_to_ldweights(nc.m)
    _bass_rust.generate_event_semaphores(nc)
    return nc


def prepare_in_maps(inputs):
    q = np.asarray(inputs["query"], np.float32)
    ipw = np.asarray(inputs["in_proj_weight"], np.float32)
    ipb = np.asarray(inputs["in_proj_bias"], np.float32)
    out_w = np.asarray(inputs["out_w"], np.float32)
    k_a = np.asarray(inputs["k_a"], np.float32)
    k_b = np.asarray(inputs["k_b"], np.float32)
    v_a = np.asarray(inputs["v_a"], np.float32)
    v_b = np.asarray(inputs["v_b"], np.float32)
    qscale = 1.0 / math.sqrt(HD)
    sl = SCALE / R

    in_maps = []
    for c in range(NCORES):
        bb = c // 4
        s = (c % 4) * CD
        e = s + CD
        X = q[:, bb, :]

        xa = np.zeros((KPAD, T), np.float32)
        xa[:D] = X.T
        xa[D] = 1.0

        wqk = np.zeros((KPAD, 2 * CD), np.float32)
        wqk[:D, :CD] = ipw[s:e].T * qscale
        wqk[D, :CD] = ipb[s:e] * qscale
        wqk[:D, CD:] = ipw[D + s : D + e].T
        wqk[D, CD:] = ipb[D + s : D + e]

        wv = np.zeros((KPAD, HPC * VW), np.float32)
        for j in range(HPC):
            wv[:D, j * VW : j * VW + HD] = ipw[2 * D + s + j * HD : 2 * D + s + (j + 1) * HD].T
            wv[D, j * VW : j * VW + HD] = ipb[2 * D + s + j * HD : 2 * D + s + (j + 1) * HD]
            wv[D, j * VW + HD] = 1.0

        ab = np.zeros((KPAD, 3 * R), np.float32)
        ab[:D, :R] = k_a.T
        ab[:D, 2 * R :] = v_a.T

        kbm = k_b[:, s:e] * sl

        vbm = np.zeros((R, HPC * VW), np.float32)
        for j in range(HPC):
            vbm[:, j * VW : j * VW + HD] = v_b[:, s + j * HD : s + (j + 1) * HD] * sl

        wo = out_w[:, s:e].T

        in_maps.append(
            {
                "xa": xa.astype(BF16),
                "wqk": wqk.astype(BF16),
                "wv": wv.astype(BF16),
                "ab": ab.astype(BF16),
                "kbm": kbm.astype(BF16),
                "vbm": vbm.astype(BF16),
                "wo": wo.astype(BF16),
            }
        )
    return in_maps


def assemble_output(inputs, results):
    out_b = np.asarray(inputs["out_b"], np.float32)
    out = np.zeros((T, BSZ, D), np.float32)
    for c in range(NCORES):
        out[:, c // 4, :] += results[c]["out"]
    out += out_b[None, None, :]
    return out


def kernel(**inputs):
    nc = build_nc()
    in_maps = prepare_in_maps(inputs)
    res = run_bass_kernel_spmd(nc, in_maps, core_ids=list(range(NCORES)))
    return assemble_output(inputs, res.results)


# revision 14
# speedup vs baseline: 1.4844x; 1.0019x over previous
"""LoRA MultiheadAttention on 8 NeuronCores (Bass/Tile), v5.

Sharding: 32 (batch, head) attention slices -> 4 heads x 1 batch per core.
Cores 0-3 take batch 0, cores 4-7 batch 1; core c handles heads
(c%4)*4 .. (c%4)*4+3, i.e. a contiguous 256-wide slice of the head dims.

The PE is drain-bound on TRN2 (every matmul costs N fp32-PSUM-drain columns
at 1 col/cycle regardless of K/M), so the kernel keeps the PE instruction
stream dense end-to-end (all matmuls bf16; fp8 was tried and rejected:
e4m3's ~4% per-element noise does not average away in random GEMMs):

  prologue: A^T LoRA activations, all of Q^T/K^T, V row-tiles 0-2.
  attention: 8 single-head units x 16 tj iterations x 2 512-wide score
             chunks. 2 chunks/iter against 3 pS slots leaves a full exp of
             cross-iteration slack, so score matmuls rarely wait. The
             remaining 13 V row-tiles (paced ahead of their PV consumers)
             and the half-0 out-projection are drained into the PE stream
             as filler so exp waits never idle the PE; filler accumulates
             in a single spare PSUM bank.
  exp split: ACT (real exp) and DVE (one-op Schraudolph bf16:
             i16 = rint(s*128/ln2 + B) bitcast bf16, mean-zero calibrated,
             ~40% of chunks; softmax renormalizes, output err ~0.5%).
  norm:      po evacuated to bf16 SBUF immediately (frees PSUM banks);
             denominator row round-trips through DRAM reshaped to [128, 8]
             so the reciprocal uses 128 DVE lanes (0.13us vs 6.5us for a
             [64,1024] broadcast reciprocal); stride-0 DMA broadcast; one
             2x-mode bf16 multiply into oT_sb.
  epilogue:  half-1 out-projection, PSUM->SBUF copies alternating ACT/DVE.

b_v is folded into the V matmul ones-row bias; out_b added on host.
"""

import sys

sys.path.insert(0, "/opt/trn_rl_repo")

import math
from contextlib import ExitStack

import ml_dtypes
import numpy as np

import concourse.bass as bass
import concourse.tile as tile
from concourse import mybir
from concourse.alu_op_type import AluOpType
from concourse.bass_utils import run_bass_kernel_spmd

BF16 = ml_dtypes.bfloat16
F32 = mybir.dt.float32
BF = mybir.dt.bfloat16
I16 = mybir.dt.int16

T = 2048
D = 1024
H = 16
HD = 64
R = 16
BSZ = 2
SCALE = 16.0
NCORES = 8
HPC = 4  # heads per core
CD = HPC * HD  # 256 head dims per core
VW = HD + 1  # V block width per head (ones column appended)
KPAD = 1152  # 1024 X rows + 1 ones row, padded to 9 k-tiles of 128
NKT = KPAD // 128
P = 128
NTT = T // P  # 16 row tiles
HF = T // 2  # 1024: ti processed in two halves

# Schraudolph-bf16 exp: i16 = rint(x * 128/ln2 + (127*128 - C)), bitcast bf16
EXP_A = 128.0 / math.log(2.0)
EXP_B = 127.0 * 128.0 - 7.3
# tj tiles whose c==1 exp chunk goes to DVE-Schraudolph (rest go to ACT)
DVE_TJ = frozenset(range(16)) - {5, 10, 15}


def build_nc():
    nc = bass.Bass()
    xa = nc.dram_tensor("xa", [KPAD, T], BF, kind="ExternalInput")
    wqk = nc.dram_tensor("wqk", [KPAD, 2 * CD], BF, kind="ExternalInput")
    wv = nc.dram_tensor("wv", [KPAD, HPC * VW], BF, kind="ExternalInput")
    ab = nc.dram_tensor("ab", [KPAD, 3 * R], BF, kind="ExternalInput")
    kbm = nc.dram_tensor("kbm", [R, CD], BF, kind="ExternalInput")
    vbm = nc.dram_tensor("vbm", [R, HPC * VW], BF, kind="ExternalInput")
    wo = nc.dram_tensor("wo", [CD, D], BF, kind="ExternalInput")
    out = nc.dram_tensor("out", [T, D], F32, kind="ExternalOutput")

    with tile.TileContext(nc) as tc, ExitStack() as ctx:
        singles = ctx.enter_context(tc.tile_pool(name="singles", bufs=1))

        xa_t = [singles.tile([P, T], BF, name=f"xa{i}", tag=f"xa{i}") for i in range(NKT)]
        wqk_t = [singles.tile([P, 2 * CD], BF, name=f"wqk{i}", tag=f"wqk{i}") for i in range(NKT)]
        wv_t = [singles.tile([P, HPC * VW], BF, name=f"wv{i}", tag=f"wv{i}") for i in range(NKT)]
        ab_t = [singles.tile([P, 3 * R], BF, name=f"ab{i}", tag=f"ab{i}") for i in range(NKT)]
        kb_t = singles.tile([R, CD], BF, tag="kb")
        vb_t = singles.tile([R, HPC * VW], BF, tag="vb")
        wo_t = [singles.tile([P, D], BF, name=f"wo{i}", tag=f"wo{i}") for i in range(2)]
        # load order matches consumption: A needs ab+xa, then B needs wqk
        for i in range(NKT):
            nc.sync.dma_start(out=ab_t[i], in_=ab[i * P : (i + 1) * P, :])
            nc.sync.dma_start(out=xa_t[i], in_=xa[i * P : (i + 1) * P, :])
        for i in range(NKT):
            nc.sync.dma_start(out=wqk_t[i], in_=wqk[i * P : (i + 1) * P, :])
        for i in range(NKT):
            nc.sync.dma_start(out=wv_t[i], in_=wv[i * P : (i + 1) * P, :])
        nc.sync.dma_start(out=kb_t, in_=kbm[:, :])
        nc.sync.dma_start(out=vb_t, in_=vbm[:, :])
        for i in range(2):
            nc.sync.dma_start(out=wo_t[i], in_=wo[i * P : (i + 1) * P, :])

        qk_sb = [singles.tile([P, T], BF, name=f"qk{i}", tag=f"qk{i}") for i in range(4)]
        ak_sb = singles.tile([R, T], BF, tag="ak")
        av_sb = singles.tile([R, T], BF, tag="av")
        v_sb = [singles.tile([P, HPC * VW], BF, name=f"v{i}", tag=f"v{i}") for i in range(NTT)]
        oT_sb = [singles.tile([P, T], BF, name=f"oT{i}", tag=f"oT{i}") for i in range(2)]
        pOut = ctx.enter_context(tc.tile_pool(name="pOut", bufs=3))
        ob_sb = {}

        def emit_b_chunk(pool, m, ch, copy_engine):
            cs = slice(ch * 512, (ch + 1) * 512)
            pq = pool.tile([P, 512], F32, tag="aux", name=f"pq_{m}_{ch}")
            steps = []
            for kt in range(NKT):
                steps.append(
                    lambda kt=kt, pq=pq: nc.tensor.matmul(
                        pq,
                        lhsT=wqk_t[kt][:, m * P : (m + 1) * P],
                        rhs=xa_t[kt][:, cs],
                        start=(kt == 0),
                        stop=(kt == NKT - 1 and m < 2),
                    )
                )
            if m >= 2:
                steps.append(
                    lambda pq=pq: nc.tensor.matmul(
                        pq,
                        lhsT=kb_t[:, (m - 2) * P : (m - 1) * P],
                        rhs=ak_sb[:, cs],
                        start=False,
                        stop=True,
                    )
                )
            steps.append(lambda pq=pq: copy_engine(qk_sb[m][:, cs], pq))
            return steps

        def emit_c_chunk(pool, mt, copy_engine):
            ms = slice(mt * P, (mt + 1) * P)
            pv = pool.tile([P, HPC * VW], F32, tag="aux", name=f"pv_{mt}")
            steps = []
            for kt in range(NKT):
                steps.append(
                    lambda kt=kt, pv=pv: nc.tensor.matmul(
                        pv,
                        lhsT=xa_t[kt][:, ms],
                        rhs=wv_t[kt],
                        start=(kt == 0),
                        stop=False,
                    )
                )
            steps.append(
                lambda pv=pv: nc.tensor.matmul(
                    pv, lhsT=av_sb[:, ms], rhs=vb_t, start=False, stop=True
                )
            )
            steps.append(lambda pv=pv: copy_engine(v_sb[mt], pv))
            return steps

        def emit_outproj_chunk(pool, mt, ch, copy_engine):
            ms = slice(mt * P, (mt + 1) * P)
            cs = slice(ch * 512, (ch + 1) * 512)
            po2 = pool.tile([P, 512], F32, tag="aux", name=f"po2_{mt}_{ch}")
            steps = []
            if ch == 0:
                def mkob(mt=mt):
                    ob_sb[mt] = pOut.tile([P, D], F32, tag="ob", name=f"ob_{mt}")
                steps.append(mkob)
            for kt2 in range(2):
                steps.append(
                    lambda kt2=kt2, po2=po2: nc.tensor.matmul(
                        po2,
                        lhsT=oT_sb[kt2][:, ms],
                        rhs=wo_t[kt2][:, cs],
                        start=(kt2 == 0),
                        stop=(kt2 == 1),
                    )
                )

            def fin(po2=po2, mt=mt, ch=ch):
                copy_engine(ob_sb[mt][:, cs], po2)
                if ch == 1:
                    nc.sync.dma_start(out=out[ms, :], in_=ob_sb[mt])

            steps.append(fin)
            return steps

        # Phase A: A_kv^T = [k_a; v_a] @ X   (48, T)
        with tc.tile_pool(name="pA", bufs=2, space="PSUM") as pA:
            for ch in range(4):
                cs = slice(ch * 512, (ch + 1) * 512)
                pa = pA.tile([3 * R, 512], F32, tag="pa")
                for kt in range(8):  # ab rows >= 1024 are zero; skip 9th tile
                    nc.tensor.matmul(
                        pa,
                        lhsT=ab_t[kt],
                        rhs=xa_t[kt][:, cs],
                        start=(kt == 0),
                        stop=(kt == 7),
                    )
                nc.vector.tensor_copy(ak_sb[:, cs], pa[0:R, :])
                nc.vector.tensor_copy(av_sb[:, cs], pa[2 * R : 3 * R, :])

        # Prologue: all of Q^T/K^T, V row-tiles 0-2
        with tc.tile_pool(name="pPro", bufs=3, space="PSUM") as pPro:
            for m in (0, 2, 1, 3):
                for ch in range(4):
                    for step in emit_b_chunk(pPro, m, ch, nc.vector.tensor_copy):
                        step()
            for mt in range(3):
                for step in emit_c_chunk(pPro, mt, nc.vector.tensor_copy):
                    step()

        # Attention: 8 single-head units with interleaved filler
        with (
            tc.tile_pool(name="pS", bufs=3, space="PSUM") as pS,
            tc.tile_pool(name="pO", bufs=2, space="PSUM") as pO,
            tc.tile_pool(name="pX", bufs=1, space="PSUM") as pX,
            tc.tile_pool(name="pP", bufs=6) as pP,
            tc.tile_pool(name="pEv", bufs=3) as pEv,
            tc.tile_pool(name="pN", bufs=3) as pN,
            tc.tile_pool(name="pD", bufs=3, space="DRAM") as pD,
        ):
            filler = []

            def alt_copy(i=[0]):
                i[0] += 1
                return nc.vector.tensor_copy if i[0] % 2 else nc.scalar.copy

            for mt in range(3, NTT):
                filler.extend(emit_c_chunk(pX, mt, alt_copy()))
            fill_pos = [0]

            def drain_filler(n):
                i = fill_pos[0]
                for _ in range(n):
                    if i >= len(filler):
                        break
                    filler[i]()
                    i += 1
                fill_pos[0] = i

            def emit_unit(half, h, fill_rate):
                hp = h // 2
                rs = slice((h % 2) * HD, (h % 2) * HD + HD)
                po = pO.tile([VW, HF], F32, tag="po", name=f"po_{half}_{h}")
                pts = {}

                def emit_pv(t):
                    pt = pts.pop(t)
                    for c in range(2):
                        nc.tensor.matmul(
                            po[:, c * 512 : (c + 1) * 512],
                            lhsT=v_sb[t][:, h * VW : (h + 1) * VW],
                            rhs=pt[:, c * 512 : (c + 1) * 512],
                            start=(t == 0),
                            stop=(t == NTT - 1),
                        )

                for tj in range(NTT):
                    pt = pP.tile([P, HF], BF, tag="pt", name=f"pt_{half}_{h}_{tj}")
                    for c in range(2):
                        s_ps = pS.tile([P, 512], F32, tag="s", name=f"ps_{half}_{h}_{tj}_{c}")
                        nc.tensor.matmul(
                            s_ps,
                            lhsT=qk_sb[2 + hp][rs, tj * P : (tj + 1) * P],
                            rhs=qk_sb[hp][rs, half * HF + c * 512 : half * HF + (c + 1) * 512],
                            start=True,
                            stop=True,
                        )
                        ptc = pt[:, c * 512 : (c + 1) * 512]
                        if c == 1 and tj in DVE_TJ:
                            nc.vector.tensor_scalar(
                                ptc.bitcast(I16), s_ps, EXP_A, EXP_B,
                                AluOpType.mult, AluOpType.add,
                            )
                        else:
                            nc.scalar.activation(
                                ptc, s_ps, mybir.ActivationFunctionType.Exp
                            )
                    pts[tj] = pt
                    drain_filler(fill_rate)
                    if tj > 0:
                        emit_pv(tj - 1)
                emit_pv(NTT - 1)
                return po

            def emit_norm(half, h, po):
                hs = slice(half * HF, (half + 1) * HF)
                ev = pEv.tile([VW, HF], BF, tag="ev", name=f"ev_{half}_{h}")
                nc.vector.tensor_copy(ev, po)
                dr = pD.tile([1, HF], BF, tag="dr", name=f"dr_{half}_{h}")
                nc.sync.dma_start(out=dr, in_=ev[HD:VW, :])
                den128 = pN.tile([P, HF // P], BF, tag="d128", name=f"d128_{half}_{h}")
                nc.sync.dma_start(
                    out=den128,
                    in_=bass.AP(tensor=dr.tensor, offset=dr.offset,
                                ap=[[HF // P, P], [1, HF // P]]),
                )
                rec = pN.tile([P, HF // P], BF, tag="rec", name=f"rec_{half}_{h}")
                with nc.allow_low_precision(
                    reason="softmax denom ~2048; bf16 recip adds ~0.4% row scale noise"
                ):
                    nc.vector.reciprocal(rec, den128)
                rr = pD.tile([1, HF], BF, tag="rr", name=f"rr_{half}_{h}")
                nc.sync.dma_start(
                    out=bass.AP(tensor=rr.tensor, offset=rr.offset,
                                ap=[[HF // P, P], [1, HF // P]]),
                    in_=rec,
                )
                rb = pN.tile([HD, HF], BF, tag="rb", name=f"rb_{half}_{h}")
                nc.sync.dma_start(
                    out=rb,
                    in_=bass.AP(tensor=rr.tensor, offset=rr.offset,
                                ap=[[0, HD], [1, HF]]),
                )
                nc.vector.tensor_mul(
                    oT_sb[h // 2][(h % 2) * HD : (h % 2) * HD + HD, hs],
                    ev[0:HD, :],
                    rb,
                )

            prev = None
            for half in range(2):
                for h in range(HPC):
                    rate = {(0, 0): 11, (0, 1): 6}.get((half, h), 2)
                    po = emit_unit(half, h, rate)
                    if prev is not None:
                        emit_norm(*prev)
                        if (half, h) == (1, 0):
                            for mt in range(NTT // 2):
                                for ch in range(2):
                                    filler.extend(
                                        emit_outproj_chunk(pX, mt, ch, alt_copy())
                                    )
                    prev = (half, h, po)
            emit_norm(*prev)
            drain_filler(len(filler))

        # Epilogue: half-1 out-projection
        with tc.tile_pool(name="pE", bufs=2, space="PSUM") as pE:
            eng = [nc.vector.tensor_copy, nc.scalar.copy]
            for mt in range(NTT // 2, NTT):
                for ch in range(2):
                    for step in emit_outproj_chunk(pE, mt, ch, eng[(mt + ch) % 2]):
                        step()

    import bass_rust as _bass_rust

    _bass_rust.move_matmul_waits_to_ldweights(nc.m)
    _bass_rust.generate_event_semaphores(nc)
    return nc


def prepare_in_maps(inputs):
    q = np.asarray(inputs["query"], np.float32)
    ipw = np.asarray(inputs["in_proj_weight"], np.float32)
    ipb = np.asarray(inputs["in_proj_bias"], np.float32)
    out_w = np.asarray(inputs["out_w"], np.float32)
    k_a = np.asarray(inputs["k_a"], np.float32)
    k_b = np.asarray(inputs["k_b"], np.float32)
    v_a = np.asarray(inputs["v_a"], np.float32)
    v_b = np.asarray(inputs["v_b"], np.float32)
    qscale = 1.0 / math.sqrt(HD)
    sl = SCALE / R

    in_maps = []
    for c in range(NCORES):
        bb = c // 4
        s = (c % 4) * CD
        e = s + CD
        X = q[:, bb, :]

        xa = np.zeros((KPAD, T), np.float32)
        xa[:D] = X.T
        xa[D] = 1.0

        wqk = np.zeros((KPAD, 2 * CD), np.float32)
        wqk[:D, :CD] = ipw[s:e].T * qscale
        wqk[D, :CD] = ipb[s:e] * qscale
        wqk[:D, CD:] = ipw[D + s : D + e].T
        wqk[D, CD:] = ipb[D + s : D + e]

        wv = np.zeros((KPAD, HPC * VW), np.float32)
        for j in range(HPC):
            wv[:D, j * VW : j * VW + HD] = ipw[2 * D + s + j * HD : 2 * D + s + (j + 1) * HD].T
            wv[D, j * VW : j * VW + HD] = ipb[2 * D + s + j * HD : 2 * D + s + (j + 1) * HD]
            wv[D, j * VW + HD] = 1.0

        ab = np.zeros((KPAD, 3 * R), np.float32)
        ab[:D, :R] = k_a.T
        ab[:D, 2 * R :] = v_a.T

        kbm = k_b[:, s:e] * sl

        vbm = np.zeros((R, HPC * VW), np.float32)
        for j in range(HPC):
            vbm[:, j * VW : j * VW + HD] = v_b[:, s + j * HD : s + (j + 1) * HD] * sl

        wo = out_w[:, s:e].T

        in_maps.append(
            {
                "xa": xa.astype(BF16),
                "wqk": wqk.astype(BF16),
                "wv": wv.astype(BF16),
                "ab": ab.astype(BF16),
                "kbm": kbm.astype(BF16),
                "vbm": vbm.astype(BF16),
                "wo": wo.astype(BF16),
            }
        )
    return in_maps


def assemble_output(inputs, results):
    out_b = np.asarray(inputs["out_b"], np.float32)
    out = np.zeros((T, BSZ, D), np.float32)
    for c in range(NCORES):
        out[:, c // 4, :] += results[c]["out"]
    out += out_b[None, None, :]
    return out


def kernel(**inputs):
    nc = build_nc()
    in_maps = prepare_in_maps(inputs)
    res = run_bass_kernel_spmd(nc, in_maps, core_ids=list(range(NCORES)))
    return assemble_output(inputs, res.results)


# revision 15
# speedup vs baseline: 1.5109x; 1.0179x over previous
"""LoRA MultiheadAttention on 8 NeuronCores (Bass/Tile), v5.

Sharding: 32 (batch, head) attention slices -> 4 heads x 1 batch per core.
Cores 0-3 take batch 0, cores 4-7 batch 1; core c handles heads
(c%4)*4 .. (c%4)*4+3, i.e. a contiguous 256-wide slice of the head dims.

The PE is drain-bound on TRN2 (every matmul costs N fp32-PSUM-drain columns
at 1 col/cycle regardless of K/M), so the kernel keeps the PE instruction
stream dense end-to-end (all matmuls bf16; fp8 was tried and rejected:
e4m3's ~4% per-element noise does not average away in random GEMMs):

  prologue: A^T LoRA activations, all of Q^T/K^T, V row-tiles 0-2.
  attention: 8 single-head units x 16 tj iterations x 2 512-wide score
             chunks. 2 chunks/iter against 3 pS slots leaves a full exp of
             cross-iteration slack, so score matmuls rarely wait. The
             remaining 13 V row-tiles (paced ahead of their PV consumers)
             and the half-0 out-projection are drained into the PE stream
             as filler so exp waits never idle the PE; filler accumulates
             in a single spare PSUM bank.
  exp split: ACT (real exp) and DVE (one-op Schraudolph bf16:
             i16 = rint(s*128/ln2 + B) bitcast bf16, mean-zero calibrated,
             ~40% of chunks; softmax renormalizes, output err ~0.5%).
  norm:      po evacuated to bf16 SBUF immediately (frees PSUM banks);
             denominator row round-trips through DRAM reshaped to [128, 8]
             so the reciprocal uses 128 DVE lanes (0.13us vs 6.5us for a
             [64,1024] broadcast reciprocal); stride-0 DMA broadcast; one
             2x-mode bf16 multiply into oT_sb.
  epilogue:  half-1 out-projection, PSUM->SBUF copies alternating ACT/DVE.

b_v is folded into the V matmul ones-row bias; out_b added on host.
"""

import sys

sys.path.insert(0, "/opt/trn_rl_repo")

import math
from contextlib import ExitStack

import ml_dtypes
import numpy as np

import concourse.bass as bass
import concourse.tile as tile
from concourse import mybir
from concourse.alu_op_type import AluOpType
from concourse.bass_utils import run_bass_kernel_spmd

BF16 = ml_dtypes.bfloat16
F32 = mybir.dt.float32
BF = mybir.dt.bfloat16
I16 = mybir.dt.int16

T = 2048
D = 1024
H = 16
HD = 64
R = 16
BSZ = 2
SCALE = 16.0
NCORES = 8
HPC = 4  # heads per core
CD = HPC * HD  # 256 head dims per core
VW = HD + 1  # V block width per head (ones column appended)
KPAD = 1152  # 1024 X rows + 1 ones row, padded to 9 k-tiles of 128
NKT = KPAD // 128
P = 128
NTT = T // P  # 16 row tiles
HF = T // 2  # 1024: ti processed in two halves

# Schraudolph-bf16 exp: i16 = rint(x * 128/ln2 + (127*128 - C)), bitcast bf16
EXP_A = 128.0 / math.log(2.0)
EXP_B = 127.0 * 128.0 - 7.3
# tj tiles whose c==1 exp chunk goes to DVE-Schraudolph (rest go to ACT)
DVE_TJ = frozenset(range(16)) - {5, 10, 15}


def build_nc():
    nc = bass.Bass()
    xa = nc.dram_tensor("xa", [KPAD, T], BF, kind="ExternalInput")
    wqk = nc.dram_tensor("wqk", [KPAD, 2 * CD], BF, kind="ExternalInput")
    wv = nc.dram_tensor("wv", [KPAD, HPC * VW], BF, kind="ExternalInput")
    ab = nc.dram_tensor("ab", [KPAD, 3 * R], BF, kind="ExternalInput")
    kbm = nc.dram_tensor("kbm", [R, CD], BF, kind="ExternalInput")
    vbm = nc.dram_tensor("vbm", [R, HPC * VW], BF, kind="ExternalInput")
    wo = nc.dram_tensor("wo", [CD, D], BF, kind="ExternalInput")
    out = nc.dram_tensor("out", [T, D], F32, kind="ExternalOutput")

    with tile.TileContext(nc) as tc, ExitStack() as ctx:
        singles = ctx.enter_context(tc.tile_pool(name="singles", bufs=1))

        xa_t = [singles.tile([P, T], BF, name=f"xa{i}", tag=f"xa{i}") for i in range(NKT)]
        wqk_t = [singles.tile([P, 2 * CD], BF, name=f"wqk{i}", tag=f"wqk{i}") for i in range(NKT)]
        wv_t = [singles.tile([P, HPC * VW], BF, name=f"wv{i}", tag=f"wv{i}") for i in range(NKT)]
        ab_t = [singles.tile([P, 3 * R], BF, name=f"ab{i}", tag=f"ab{i}") for i in range(NKT)]
        kb_t = singles.tile([R, CD], BF, tag="kb")
        vb_t = singles.tile([R, HPC * VW], BF, tag="vb")
        wo_t = [singles.tile([P, D], BF, name=f"wo{i}", tag=f"wo{i}") for i in range(2)]
        # load order matches consumption: A needs ab+xa, then B needs wqk
        for i in range(NKT):
            nc.sync.dma_start(out=ab_t[i], in_=ab[i * P : (i + 1) * P, :])
            nc.sync.dma_start(out=xa_t[i], in_=xa[i * P : (i + 1) * P, :])
        for i in range(NKT):
            nc.sync.dma_start(out=wqk_t[i], in_=wqk[i * P : (i + 1) * P, :])
        for i in range(NKT):
            nc.sync.dma_start(out=wv_t[i], in_=wv[i * P : (i + 1) * P, :])
        nc.sync.dma_start(out=kb_t, in_=kbm[:, :])
        nc.sync.dma_start(out=vb_t, in_=vbm[:, :])
        for i in range(2):
            nc.sync.dma_start(out=wo_t[i], in_=wo[i * P : (i + 1) * P, :])

        qk_sb = [singles.tile([P, T], BF, name=f"qk{i}", tag=f"qk{i}") for i in range(4)]
        ak_sb = singles.tile([R, T], BF, tag="ak")
        av_sb = singles.tile([R, T], BF, tag="av")
        v_sb = [singles.tile([P, HPC * VW], BF, name=f"v{i}", tag=f"v{i}") for i in range(NTT)]
        oT_sb = [singles.tile([P, T], BF, name=f"oT{i}", tag=f"oT{i}") for i in range(2)]
        pOut = ctx.enter_context(tc.tile_pool(name="pOut", bufs=3))
        ob_sb = {}

        def emit_b_chunk(pool, m, ch, copy_engine):
            cs = slice(ch * 512, (ch + 1) * 512)
            pq = pool.tile([P, 512], F32, tag="aux", name=f"pq_{m}_{ch}")
            steps = []
            for kt in range(NKT):
                steps.append(
                    lambda kt=kt, pq=pq: nc.tensor.matmul(
                        pq,
                        lhsT=wqk_t[kt][:, m * P : (m + 1) * P],
                        rhs=xa_t[kt][:, cs],
                        start=(kt == 0),
                        stop=(kt == NKT - 1 and m < 2),
                    )
                )
            if m >= 2:
                steps.append(
                    lambda pq=pq: nc.tensor.matmul(
                        pq,
                        lhsT=kb_t[:, (m - 2) * P : (m - 1) * P],
                        rhs=ak_sb[:, cs],
                        start=False,
                        stop=True,
                    )
                )
            steps.append(lambda pq=pq: copy_engine(qk_sb[m][:, cs], pq))
            return steps

        def emit_c_chunk(pool, mt, copy_engine):
            ms = slice(mt * P, (mt + 1) * P)
            pv = pool.tile([P, HPC * VW], F32, tag="aux", name=f"pv_{mt}")
            steps = []
            for kt in range(NKT):
                steps.append(
                    lambda kt=kt, pv=pv: nc.tensor.matmul(
                        pv,
                        lhsT=xa_t[kt][:, ms],
                        rhs=wv_t[kt],
                        start=(kt == 0),
                        stop=False,
                    )
                )
            steps.append(
                lambda pv=pv: nc.tensor.matmul(
                    pv, lhsT=av_sb[:, ms], rhs=vb_t, start=False, stop=True
                )
            )
            steps.append(lambda pv=pv: copy_engine(v_sb[mt], pv))
            return steps

        def emit_outproj_chunk(pool, mt, ch, copy_engine):
            ms = slice(mt * P, (mt + 1) * P)
            cs = slice(ch * 512, (ch + 1) * 512)
            po2 = pool.tile([P, 512], F32, tag="aux", name=f"po2_{mt}_{ch}")
            steps = []
            if ch == 0:
                def mkob(mt=mt):
                    ob_sb[mt] = pOut.tile([P, D], F32, tag="ob", name=f"ob_{mt}")
                steps.append(mkob)
            for kt2 in range(2):
                steps.append(
                    lambda kt2=kt2, po2=po2: nc.tensor.matmul(
                        po2,
                        lhsT=oT_sb[kt2][:, ms],
                        rhs=wo_t[kt2][:, cs],
                        start=(kt2 == 0),
                        stop=(kt2 == 1),
                    )
                )

            def fin(po2=po2, mt=mt, ch=ch):
                copy_engine(ob_sb[mt][:, cs], po2)
                if ch == 1:
                    nc.sync.dma_start(out=out[ms, :], in_=ob_sb[mt])

            steps.append(fin)
            return steps

        # Phase A: A_kv^T = [k_a; v_a] @ X   (48, T); kt-outer so each
        # matmul consumes an xa k-tile as soon as its DMA lands
        with tc.tile_pool(name="pA", bufs=4, space="PSUM") as pA:
            pa4 = [pA.tile([3 * R, 512], F32, tag="pa", name=f"pa{ch}") for ch in range(4)]
            for kt in range(8):  # ab rows >= 1024 are zero; skip 9th tile
                for ch in range(4):
                    nc.tensor.matmul(
                        pa4[ch],
                        lhsT=ab_t[kt],
                        rhs=xa_t[kt][:, ch * 512 : (ch + 1) * 512],
                        start=(kt == 0),
                        stop=(kt == 7),
                    )
            for ch in range(4):
                cs = slice(ch * 512, (ch + 1) * 512)
                nc.vector.tensor_copy(ak_sb[:, cs], pa4[ch][0:R, :])
                nc.vector.tensor_copy(av_sb[:, cs], pa4[ch][2 * R : 3 * R, :])

        # Prologue: all of Q^T/K^T, V row-tiles 0-2
        with tc.tile_pool(name="pPro", bufs=3, space="PSUM") as pPro:
            for m in (0, 2, 1, 3):
                for ch in range(4):
                    for step in emit_b_chunk(pPro, m, ch, nc.vector.tensor_copy):
                        step()
            for mt in range(3):
                for step in emit_c_chunk(pPro, mt, nc.vector.tensor_copy):
                    step()

        # Attention: 8 single-head units with interleaved filler
        with (
            tc.tile_pool(name="pS", bufs=4, space="PSUM") as pS,
            tc.tile_pool(name="pO", bufs=3, space="PSUM") as pO,
            tc.tile_pool(name="pX", bufs=1, space="PSUM") as pX,
            tc.tile_pool(name="pP", bufs=6) as pP,
            tc.tile_pool(name="pEv", bufs=3) as pEv,
            tc.tile_pool(name="pN", bufs=3) as pN,
            tc.tile_pool(name="pD", bufs=3, space="DRAM") as pD,
        ):
            filler = []

            def alt_copy(i=[0]):
                i[0] += 1
                return nc.vector.tensor_copy if i[0] % 2 else nc.scalar.copy

            for mt in range(3, NTT):
                filler.extend(emit_c_chunk(pX, mt, alt_copy()))
            fill_pos = [0]

            def drain_filler(n):
                i = fill_pos[0]
                for _ in range(n):
                    if i >= len(filler):
                        break
                    filler[i]()
                    i += 1
                fill_pos[0] = i

            def emit_unit(half, h, fill_rate):
                hp = h // 2
                rs = slice((h % 2) * HD, (h % 2) * HD + HD)
                po = [
                    pO.tile([VW, 512], F32, tag="po", name=f"po_{half}_{h}_{c}")
                    for c in range(2)
                ]
                pts = {}

                def emit_pv(t):
                    pt = pts.pop(t)
                    for c in range(2):
                        nc.tensor.matmul(
                            po[c],
                            lhsT=v_sb[t][:, h * VW : (h + 1) * VW],
                            rhs=pt[:, c * 512 : (c + 1) * 512],
                            start=(t == 0),
                            stop=(t == NTT - 1),
                        )

                for tj in range(NTT):
                    pt = pP.tile([P, HF], BF, tag="pt", name=f"pt_{half}_{h}_{tj}")
                    for c in range(2):
                        s_ps = pS.tile([P, 512], F32, tag="s", name=f"ps_{half}_{h}_{tj}_{c}")
                        nc.tensor.matmul(
                            s_ps,
                            lhsT=qk_sb[2 + hp][rs, tj * P : (tj + 1) * P],
                            rhs=qk_sb[hp][rs, half * HF + c * 512 : half * HF + (c + 1) * 512],
                            start=True,
                            stop=True,
                        )
                        ptc = pt[:, c * 512 : (c + 1) * 512]
                        if c == 1 and tj in DVE_TJ:
                            nc.vector.tensor_scalar(
                                ptc.bitcast(I16), s_ps, EXP_A, EXP_B,
                                AluOpType.mult, AluOpType.add,
                            )
                        else:
                            nc.scalar.activation(
                                ptc, s_ps, mybir.ActivationFunctionType.Exp
                            )
                    pts[tj] = pt
                    drain_filler(fill_rate)
                    if tj > 1:
                        emit_pv(tj - 2)
                emit_pv(NTT - 2)
                emit_pv(NTT - 1)
                return po

            def emit_norm(half, h, po):
                hs = slice(half * HF, (half + 1) * HF)
                ev = pEv.tile([VW, HF], BF, tag="ev", name=f"ev_{half}_{h}")
                for c in range(2):
                    nc.vector.tensor_copy(ev[:, c * 512 : (c + 1) * 512], po[c])
                dr = pD.tile([1, HF], BF, tag="dr", name=f"dr_{half}_{h}")
                nc.sync.dma_start(out=dr, in_=ev[HD:VW, :])
                den128 = pN.tile([P, HF // P], BF, tag="d128", name=f"d128_{half}_{h}")
                nc.sync.dma_start(
                    out=den128,
                    in_=bass.AP(tensor=dr.tensor, offset=dr.offset,
                                ap=[[HF // P, P], [1, HF // P]]),
                )
                rec = pN.tile([P, HF // P], BF, tag="rec", name=f"rec_{half}_{h}")
                with nc.allow_low_precision(
                    reason="softmax denom ~2048; bf16 recip adds ~0.4% row scale noise"
                ):
                    nc.vector.reciprocal(rec, den128)
                rr = pD.tile([1, HF], BF, tag="rr", name=f"rr_{half}_{h}")
                nc.sync.dma_start(
                    out=bass.AP(tensor=rr.tensor, offset=rr.offset,
                                ap=[[HF // P, P], [1, HF // P]]),
                    in_=rec,
                )
                rb = pN.tile([HD, HF], BF, tag="rb", name=f"rb_{half}_{h}")
                nc.sync.dma_start(
                    out=rb,
                    in_=bass.AP(tensor=rr.tensor, offset=rr.offset,
                                ap=[[0, HD], [1, HF]]),
                )
                nc.vector.tensor_mul(
                    oT_sb[h // 2][(h % 2) * HD : (h % 2) * HD + HD, hs],
                    ev[0:HD, :],
                    rb,
                )

            prev = None
            for half in range(2):
                for h in range(HPC):
                    rate = {(0, 0): 11, (0, 1): 6}.get((half, h), 2)
                    po = emit_unit(half, h, rate)
                    if prev is not None:
                        emit_norm(*prev)
                        if (half, h) == (1, 0):
                            for mt in range(NTT // 2):
                                for ch in range(2):
                                    filler.extend(
                                        emit_outproj_chunk(pX, mt, ch, alt_copy())
                                    )
                    prev = (half, h, po)
            emit_norm(*prev)
            drain_filler(len(filler))

        # Epilogue: half-1 out-projection
        with tc.tile_pool(name="pE", bufs=2, space="PSUM") as pE:
            eng = [nc.vector.tensor_copy, nc.scalar.copy]
            for mt in range(NTT // 2, NTT):
                for ch in range(2):
                    for step in emit_outproj_chunk(pE, mt, ch, eng[(mt + ch) % 2]):
                        step()

    import bass_rust as _bass_rust

    _bass_rust.move_matmul_waits_to_ldweights(nc.m)
    _bass_rust.generate_event_semaphores(nc)
    return nc


def prepare_in_maps(inputs):
    q = np.asarray(inputs["query"], np.float32)
    ipw = np.asarray(inputs["in_proj_weight"], np.float32)
    ipb = np.asarray(inputs["in_proj_bias"], np.float32)
    out_w = np.asarray(inputs["out_w"], np.float32)
    k_a = np.asarray(inputs["k_a"], np.float32)
    k_b = np.asarray(inputs["k_b"], np.float32)
    v_a = np.asarray(inputs["v_a"], np.float32)
    v_b = np.asarray(inputs["v_b"], np.float32)
    qscale = 1.0 / math.sqrt(HD)
    sl = SCALE / R

    in_maps = []
    for c in range(NCORES):
        bb = c // 4
        s = (c % 4) * CD
        e = s + CD
        X = q[:, bb, :]

        xa = np.zeros((KPAD, T), np.float32)
        xa[:D] = X.T
        xa[D] = 1.0

        wqk = np.zeros((KPAD, 2 * CD), np.float32)
        wqk[:D, :CD] = ipw[s:e].T * qscale
        wqk[D, :CD] = ipb[s:e] * qscale
        wqk[:D, CD:] = ipw[D + s : D + e].T
        wqk[D, CD:] = ipb[D + s : D + e]

        wv = np.zeros((KPAD, HPC * VW), np.float32)
        for j in range(HPC):
            wv[:D, j * VW : j * VW + HD] = ipw[2 * D + s + j * HD : 2 * D + s + (j + 1) * HD].T
            wv[D, j * VW : j * VW + HD] = ipb[2 * D + s + j * HD : 2 * D + s + (j + 1) * HD]
            wv[D, j * VW + HD] = 1.0

        ab = np.zeros((KPAD, 3 * R), np.float32)
        ab[:D, :R] = k_a.T
        ab[:D, 2 * R :] = v_a.T

        kbm = k_b[:, s:e] * sl

        vbm = np.zeros((R, HPC * VW), np.float32)
        for j in range(HPC):
            vbm[:, j * VW : j * VW + HD] = v_b[:, s + j * HD : s + (j + 1) * HD] * sl

        wo = out_w[:, s:e].T

        in_maps.append(
            {
                "xa": xa.astype(BF16),
                "wqk": wqk.astype(BF16),
                "wv": wv.astype(BF16),
                "ab": ab.astype(BF16),
                "kbm": kbm.astype(BF16),
                "vbm": vbm.astype(BF16),
                "wo": wo.astype(BF16),
            }
        )
    return in_maps


def assemble_output(inputs, results):
    out_b = np.asarray(inputs["out_b"], np.float32)
    out = np.zeros((T, BSZ, D), np.float32)
    for c in range(NCORES):
        out[:, c // 4, :] += results[c]["out"]
    out += out_b[None, None, :]
    return out


def kernel(**inputs):
    nc = build_nc()
    in_maps = prepare_in_maps(inputs)
    res = run_bass_kernel_spmd(nc, in_maps, core_ids=list(range(NCORES)))
    return assemble_output(inputs, res.results)


# revision 16
# speedup vs baseline: 1.5252x; 1.0095x over previous
"""LoRA MultiheadAttention on 8 NeuronCores (Bass/Tile), v5.

Sharding: 32 (batch, head) attention slices -> 4 heads x 1 batch per core.
Cores 0-3 take batch 0, cores 4-7 batch 1; core c handles heads
(c%4)*4 .. (c%4)*4+3, i.e. a contiguous 256-wide slice of the head dims.

The PE is drain-bound on TRN2 (every matmul costs N fp32-PSUM-drain columns
at 1 col/cycle regardless of K/M), so the kernel keeps the PE instruction
stream dense end-to-end (all matmuls bf16; fp8 was tried and rejected:
e4m3's ~4% per-element noise does not average away in random GEMMs):

  prologue: A^T LoRA activations, all of Q^T/K^T, V row-tiles 0-2.
  attention: 8 single-head units x 16 tj iterations x 2 512-wide score
             chunks. 2 chunks/iter against 3 pS slots leaves a full exp of
             cross-iteration slack, so score matmuls rarely wait. The
             remaining 13 V row-tiles (paced ahead of their PV consumers)
             and the half-0 out-projection are drained into the PE stream
             as filler so exp waits never idle the PE; filler accumulates
             in a single spare PSUM bank.
  exp split: ACT (real exp) and DVE (one-op Schraudolph bf16:
             i16 = rint(s*128/ln2 + B) bitcast bf16, mean-zero calibrated,
             ~40% of chunks; softmax renormalizes, output err ~0.5%).
  norm:      po evacuated to bf16 SBUF immediately (frees PSUM banks);
             denominator row round-trips through DRAM reshaped to [128, 8]
             so the reciprocal uses 128 DVE lanes (0.13us vs 6.5us for a
             [64,1024] broadcast reciprocal); stride-0 DMA broadcast; one
             2x-mode bf16 multiply into oT_sb.
  epilogue:  half-1 out-projection, PSUM->SBUF copies alternating ACT/DVE.

b_v is folded into the V matmul ones-row bias; out_b added on host.
"""

import sys

sys.path.insert(0, "/opt/trn_rl_repo")

import math
from contextlib import ExitStack

import ml_dtypes
import numpy as np

import concourse.bass as bass
import concourse.tile as tile
from concourse import mybir
from concourse.alu_op_type import AluOpType
from concourse.bass_utils import run_bass_kernel_spmd

BF16 = ml_dtypes.bfloat16
F32 = mybir.dt.float32
BF = mybir.dt.bfloat16
I16 = mybir.dt.int16

T = 2048
D = 1024
H = 16
HD = 64
R = 16
BSZ = 2
SCALE = 16.0
NCORES = 8
HPC = 4  # heads per core
CD = HPC * HD  # 256 head dims per core
VW = HD + 1  # V block width per head (ones column appended)
KPAD = 1152  # 1024 X rows + 1 ones row, padded to 9 k-tiles of 128
NKT = KPAD // 128
P = 128
NTT = T // P  # 16 row tiles
HF = T // 2  # 1024: ti processed in two halves

# Schraudolph-bf16 exp: i16 = rint(x * 128/ln2 + (127*128 - C)), bitcast bf16
EXP_A = 128.0 / math.log(2.0)
EXP_B = 127.0 * 128.0 - 7.3
# tj tiles whose c==1 exp chunk goes to DVE-Schraudolph (rest go to ACT)
DVE_TJ = frozenset(range(16)) - {5, 10, 15}


def build_nc():
    nc = bass.Bass()
    xa = nc.dram_tensor("xa", [KPAD, T], BF, kind="ExternalInput")
    wqk = nc.dram_tensor("wqk", [KPAD, 2 * CD], BF, kind="ExternalInput")
    wv = nc.dram_tensor("wv", [KPAD, HPC * VW], BF, kind="ExternalInput")
    ab = nc.dram_tensor("ab", [KPAD, 3 * R], BF, kind="ExternalInput")
    kbm = nc.dram_tensor("kbm", [R, CD], BF, kind="ExternalInput")
    vbm = nc.dram_tensor("vbm", [R, HPC * VW], BF, kind="ExternalInput")
    wo = nc.dram_tensor("wo", [CD, D], BF, kind="ExternalInput")
    out = nc.dram_tensor("out", [T, D], F32, kind="ExternalOutput")

    with tile.TileContext(nc) as tc, ExitStack() as ctx:
        singles = ctx.enter_context(tc.tile_pool(name="singles", bufs=1))

        xa_t = [singles.tile([P, T], BF, name=f"xa{i}", tag=f"xa{i}") for i in range(NKT)]
        wqk_t = [singles.tile([P, 2 * CD], BF, name=f"wqk{i}", tag=f"wqk{i}") for i in range(NKT)]
        wv_t = [singles.tile([P, HPC * VW], BF, name=f"wv{i}", tag=f"wv{i}") for i in range(NKT)]
        ab_t = [singles.tile([P, 3 * R], BF, name=f"ab{i}", tag=f"ab{i}") for i in range(NKT)]
        kb_t = singles.tile([R, CD], BF, tag="kb")
        vb_t = singles.tile([R, HPC * VW], BF, tag="vb")
        wo_t = [singles.tile([P, D], BF, name=f"wo{i}", tag=f"wo{i}") for i in range(2)]
        # load order matches consumption: A needs ab+xa, then B needs wqk
        for i in range(NKT):
            nc.sync.dma_start(out=ab_t[i], in_=ab[i * P : (i + 1) * P, :])
            nc.sync.dma_start(out=xa_t[i], in_=xa[i * P : (i + 1) * P, :])
        for i in range(NKT):
            nc.sync.dma_start(out=wqk_t[i], in_=wqk[i * P : (i + 1) * P, :])
        for i in range(NKT):
            nc.sync.dma_start(out=wv_t[i], in_=wv[i * P : (i + 1) * P, :])
        nc.sync.dma_start(out=kb_t, in_=kbm[:, :])
        nc.sync.dma_start(out=vb_t, in_=vbm[:, :])
        for i in range(2):
            nc.sync.dma_start(out=wo_t[i], in_=wo[i * P : (i + 1) * P, :])

        qk_sb = [singles.tile([P, T], BF, name=f"qk{i}", tag=f"qk{i}") for i in range(4)]
        ak_sb = singles.tile([R, T], BF, tag="ak")
        av_sb = singles.tile([R, T], BF, tag="av")
        v_sb = [singles.tile([P, HPC * VW], BF, name=f"v{i}", tag=f"v{i}") for i in range(NTT)]
        oT_sb = [singles.tile([P, T], BF, name=f"oT{i}", tag=f"oT{i}") for i in range(2)]
        pOut = ctx.enter_context(tc.tile_pool(name="pOut", bufs=3))
        ob_sb = {}

        def emit_b_chunk(pool, m, ch, copy_engine):
            cs = slice(ch * 512, (ch + 1) * 512)
            pq = pool.tile([P, 512], F32, tag="aux", name=f"pq_{m}_{ch}")
            steps = []
            for kt in range(NKT):
                steps.append(
                    lambda kt=kt, pq=pq: nc.tensor.matmul(
                        pq,
                        lhsT=wqk_t[kt][:, m * P : (m + 1) * P],
                        rhs=xa_t[kt][:, cs],
                        start=(kt == 0),
                        stop=(kt == NKT - 1 and m < 2),
                    )
                )
            if m >= 2:
                steps.append(
                    lambda pq=pq: nc.tensor.matmul(
                        pq,
                        lhsT=kb_t[:, (m - 2) * P : (m - 1) * P],
                        rhs=ak_sb[:, cs],
                        start=False,
                        stop=True,
                    )
                )
            steps.append(lambda pq=pq: copy_engine(qk_sb[m][:, cs], pq))
            return steps

        def emit_c_chunk(pool, mt, copy_engine):
            ms = slice(mt * P, (mt + 1) * P)
            pv = pool.tile([P, HPC * VW], F32, tag="aux", name=f"pv_{mt}")
            steps = []
            for kt in range(NKT):
                steps.append(
                    lambda kt=kt, pv=pv: nc.tensor.matmul(
                        pv,
                        lhsT=xa_t[kt][:, ms],
                        rhs=wv_t[kt],
                        start=(kt == 0),
                        stop=False,
                    )
                )
            steps.append(
                lambda pv=pv: nc.tensor.matmul(
                    pv, lhsT=av_sb[:, ms], rhs=vb_t, start=False, stop=True
                )
            )
            steps.append(lambda pv=pv: copy_engine(v_sb[mt], pv))
            return steps

        def emit_outproj_chunk(pool, mt, ch, copy_engine):
            ms = slice(mt * P, (mt + 1) * P)
            cs = slice(ch * 512, (ch + 1) * 512)
            po2 = pool.tile([P, 512], F32, tag="aux", name=f"po2_{mt}_{ch}")
            steps = []
            if ch == 0:
                def mkob(mt=mt):
                    ob_sb[mt] = pOut.tile([P, D], F32, tag="ob", name=f"ob_{mt}")
                steps.append(mkob)
            for kt2 in range(2):
                steps.append(
                    lambda kt2=kt2, po2=po2: nc.tensor.matmul(
                        po2,
                        lhsT=oT_sb[kt2][:, ms],
                        rhs=wo_t[kt2][:, cs],
                        start=(kt2 == 0),
                        stop=(kt2 == 1),
                    )
                )

            def fin(po2=po2, mt=mt, ch=ch):
                copy_engine(ob_sb[mt][:, cs], po2)
                if ch == 1:
                    nc.sync.dma_start(out=out[ms, :], in_=ob_sb[mt])

            steps.append(fin)
            return steps

        # Phase A: A_kv^T = [k_a; v_a] @ X   (48, T); kt-outer so each
        # matmul consumes an xa k-tile as soon as its DMA lands
        with tc.tile_pool(name="pA", bufs=4, space="PSUM") as pA:
            pa4 = [pA.tile([3 * R, 512], F32, tag="pa", name=f"pa{ch}") for ch in range(4)]
            for kt in range(8):  # ab rows >= 1024 are zero; skip 9th tile
                for ch in range(4):
                    nc.tensor.matmul(
                        pa4[ch],
                        lhsT=ab_t[kt],
                        rhs=xa_t[kt][:, ch * 512 : (ch + 1) * 512],
                        start=(kt == 0),
                        stop=(kt == 7),
                    )
            for ch in range(4):
                cs = slice(ch * 512, (ch + 1) * 512)
                nc.vector.tensor_copy(ak_sb[:, cs], pa4[ch][0:R, :])
                nc.vector.tensor_copy(av_sb[:, cs], pa4[ch][2 * R : 3 * R, :])

        # Prologue: just enough of Q^T/K^T for heads 0-1 (m0 half-0 cols,
        # all of m2) plus V row-tiles 0-2; the rest of B and C drains into
        # the attention stream as filler.
        with tc.tile_pool(name="pPro", bufs=3, space="PSUM") as pPro:
            for m, ch in [(2, 0), (2, 1), (2, 2), (2, 3), (0, 0), (0, 1)]:
                for step in emit_b_chunk(pPro, m, ch, nc.vector.tensor_copy):
                    step()
            for mt in range(3):
                for step in emit_c_chunk(pPro, mt, nc.vector.tensor_copy):
                    step()

        # Attention: 8 single-head units with interleaved filler
        with (
            tc.tile_pool(name="pS", bufs=4, space="PSUM") as pS,
            tc.tile_pool(name="pO", bufs=3, space="PSUM") as pO,
            tc.tile_pool(name="pX", bufs=1, space="PSUM") as pX,
            tc.tile_pool(name="pP", bufs=6) as pP,
            tc.tile_pool(name="pEv", bufs=3) as pEv,
            tc.tile_pool(name="pN", bufs=3) as pN,
            tc.tile_pool(name="pD", bufs=3, space="DRAM") as pD,
        ):
            filler = []

            def alt_copy(i=[0]):
                i[0] += 1
                return nc.vector.tensor_copy if i[0] % 2 else nc.scalar.copy

            # queue order respects consumer deadlines: C[mt] before unit
            # (0,0)'s PV(mt); m3+m1(half0) before unit (0,2); m0(half1)
            # before unit (1,0); m1(half1) before unit (1,2)
            for mt in range(3, NTT):
                filler.extend(emit_c_chunk(pX, mt, alt_copy()))
            for m, ch in [(3, 0), (3, 1), (3, 2), (3, 3), (1, 0), (1, 1),
                          (0, 2), (0, 3), (1, 2), (1, 3)]:
                filler.extend(emit_b_chunk(pX, m, ch, alt_copy()))
            fill_pos = [0]

            def drain_filler(n):
                i = fill_pos[0]
                for _ in range(n):
                    if i >= len(filler):
                        break
                    filler[i]()
                    i += 1
                fill_pos[0] = i

            def emit_unit(half, h, fill_rate):
                hp = h // 2
                rs = slice((h % 2) * HD, (h % 2) * HD + HD)
                po = [
                    pO.tile([VW, 512], F32, tag="po", name=f"po_{half}_{h}_{c}")
                    for c in range(2)
                ]
                pts = {}

                def emit_pv(t):
                    pt = pts.pop(t)
                    for c in range(2):
                        nc.tensor.matmul(
                            po[c],
                            lhsT=v_sb[t][:, h * VW : (h + 1) * VW],
                            rhs=pt[:, c * 512 : (c + 1) * 512],
                            start=(t == 0),
                            stop=(t == NTT - 1),
                        )

                for tj in range(NTT):
                    pt = pP.tile([P, HF], BF, tag="pt", name=f"pt_{half}_{h}_{tj}")
                    for c in range(2):
                        s_ps = pS.tile([P, 512], F32, tag="s", name=f"ps_{half}_{h}_{tj}_{c}")
                        nc.tensor.matmul(
                            s_ps,
                            lhsT=qk_sb[2 + hp][rs, tj * P : (tj + 1) * P],
                            rhs=qk_sb[hp][rs, half * HF + c * 512 : half * HF + (c + 1) * 512],
                            start=True,
                            stop=True,
                        )
                        ptc = pt[:, c * 512 : (c + 1) * 512]
                        if c == 1 and tj in DVE_TJ:
                            nc.vector.tensor_scalar(
                                ptc.bitcast(I16), s_ps, EXP_A, EXP_B,
                                AluOpType.mult, AluOpType.add,
                            )
                        else:
                            nc.scalar.activation(
                                ptc, s_ps, mybir.ActivationFunctionType.Exp
                            )
                    pts[tj] = pt
                    drain_filler(fill_rate)
                    if tj > 1:
                        emit_pv(tj - 2)
                emit_pv(NTT - 2)
                emit_pv(NTT - 1)
                return po

            def emit_norm(half, h, po):
                hs = slice(half * HF, (half + 1) * HF)
                ev = pEv.tile([VW, HF], BF, tag="ev", name=f"ev_{half}_{h}")
                for c in range(2):
                    nc.vector.tensor_copy(ev[:, c * 512 : (c + 1) * 512], po[c])
                dr = pD.tile([1, HF], BF, tag="dr", name=f"dr_{half}_{h}")
                nc.sync.dma_start(out=dr, in_=ev[HD:VW, :])
                den128 = pN.tile([P, HF // P], BF, tag="d128", name=f"d128_{half}_{h}")
                nc.sync.dma_start(
                    out=den128,
                    in_=bass.AP(tensor=dr.tensor, offset=dr.offset,
                                ap=[[HF // P, P], [1, HF // P]]),
                )
                rec = pN.tile([P, HF // P], BF, tag="rec", name=f"rec_{half}_{h}")
                with nc.allow_low_precision(
                    reason="softmax denom ~2048; bf16 recip adds ~0.4% row scale noise"
                ):
                    nc.vector.reciprocal(rec, den128)
                rr = pD.tile([1, HF], BF, tag="rr", name=f"rr_{half}_{h}")
                nc.sync.dma_start(
                    out=bass.AP(tensor=rr.tensor, offset=rr.offset,
                                ap=[[HF // P, P], [1, HF // P]]),
                    in_=rec,
                )
                rb = pN.tile([HD, HF], BF, tag="rb", name=f"rb_{half}_{h}")
                nc.sync.dma_start(
                    out=rb,
                    in_=bass.AP(tensor=rr.tensor, offset=rr.offset,
                                ap=[[0, HD], [1, HF]]),
                )
                nc.vector.tensor_mul(
                    oT_sb[h // 2][(h % 2) * HD : (h % 2) * HD + HD, hs],
                    ev[0:HD, :],
                    rb,
                )

            prev = None
            for half in range(2):
                for h in range(HPC):
                    rate = {(0, 0): 9, (0, 1): 6, (0, 2): 3}.get((half, h), 2)
                    po = emit_unit(half, h, rate)
                    if prev is not None:
                        emit_norm(*prev)
                        if (half, h) == (1, 0):
                            for mt in range(NTT // 2):
                                for ch in range(2):
                                    filler.extend(
                                        emit_outproj_chunk(pX, mt, ch, alt_copy())
                                    )
                    prev = (half, h, po)
            emit_norm(*prev)
            drain_filler(len(filler))

        # Epilogue: half-1 out-projection
        with tc.tile_pool(name="pE", bufs=2, space="PSUM") as pE:
            eng = [nc.vector.tensor_copy, nc.scalar.copy]
            for mt in range(NTT // 2, NTT):
                for ch in range(2):
                    for step in emit_outproj_chunk(pE, mt, ch, eng[(mt + ch) % 2]):
                        step()

    import bass_rust as _bass_rust

    _bass_rust.move_matmul_waits_to_ldweights(nc.m)
    _bass_rust.generate_event_semaphores(nc)
    return nc


def prepare_in_maps(inputs):
    q = np.asarray(inputs["query"], np.float32)
    ipw = np.asarray(inputs["in_proj_weight"], np.float32)
    ipb = np.asarray(inputs["in_proj_bias"], np.float32)
    out_w = np.asarray(inputs["out_w"], np.float32)
    k_a = np.asarray(inputs["k_a"], np.float32)
    k_b = np.asarray(inputs["k_b"], np.float32)
    v_a = np.asarray(inputs["v_a"], np.float32)
    v_b = np.asarray(inputs["v_b"], np.float32)
    qscale = 1.0 / math.sqrt(HD)
    sl = SCALE / R

    in_maps = []
    for c in range(NCORES):
        bb = c // 4
        s = (c % 4) * CD
        e = s + CD
        X = q[:, bb, :]

        xa = np.zeros((KPAD, T), np.float32)
        xa[:D] = X.T
        xa[D] = 1.0

        wqk = np.zeros((KPAD, 2 * CD), np.float32)
        wqk[:D, :CD] = ipw[s:e].T * qscale
        wqk[D, :CD] = ipb[s:e] * qscale
        wqk[:D, CD:] = ipw[D + s : D + e].T
        wqk[D, CD:] = ipb[D + s : D + e]

        wv = np.zeros((KPAD, HPC * VW), np.float32)
        for j in range(HPC):
            wv[:D, j * VW : j * VW + HD] = ipw[2 * D + s + j * HD : 2 * D + s + (j + 1) * HD].T
            wv[D, j * VW : j * VW + HD] = ipb[2 * D + s + j * HD : 2 * D + s + (j + 1) * HD]
            wv[D, j * VW + HD] = 1.0

        ab = np.zeros((KPAD, 3 * R), np.float32)
        ab[:D, :R] = k_a.T
        ab[:D, 2 * R :] = v_a.T

        kbm = k_b[:, s:e] * sl

        vbm = np.zeros((R, HPC * VW), np.float32)
        for j in range(HPC):
            vbm[:, j * VW : j * VW + HD] = v_b[:, s + j * HD : s + (j + 1) * HD] * sl

        wo = out_w[:, s:e].T

        in_maps.append(
            {
                "xa": xa.astype(BF16),
                "wqk": wqk.astype(BF16),
                "wv": wv.astype(BF16),
                "ab": ab.astype(BF16),
                "kbm": kbm.astype(BF16),
                "vbm": vbm.astype(BF16),
                "wo": wo.astype(BF16),
            }
        )
    return in_maps


def assemble_output(inputs, results):
    out_b = np.asarray(inputs["out_b"], np.float32)
    out = np.zeros((T, BSZ, D), np.float32)
    for c in range(NCORES):
        out[:, c // 4, :] += results[c]["out"]
    out += out_b[None, None, :]
    return out


def kernel(**inputs):
    nc = build_nc()
    in_maps = prepare_in_maps(inputs)
    res = run_bass_kernel_spmd(nc, in_maps, core_ids=list(range(NCORES)))
    return assemble_output(inputs, res.results)


# revision 17
# speedup vs baseline: 1.6785x; 1.1005x over previous
"""LoRA MultiheadAttention on 8 NeuronCores (Bass/Tile), v5.

Sharding: 32 (batch, head) attention slices -> 4 heads x 1 batch per core.
Cores 0-3 take batch 0, cores 4-7 batch 1; core c handles heads
(c%4)*4 .. (c%4)*4+3, i.e. a contiguous 256-wide slice of the head dims.

The PE is drain-bound on TRN2 (every matmul costs N fp32-PSUM-drain columns
at 1 col/cycle regardless of K/M), so the kernel keeps the PE instruction
stream dense end-to-end (all matmuls bf16; fp8 was tried and rejected:
e4m3's ~4% per-element noise does not average away in random GEMMs):

  prologue: A^T LoRA activations, all of Q^T/K^T, V row-tiles 0-2.
  attention: 8 single-head units x 16 tj iterations x 2 512-wide score
             chunks. 2 chunks/iter against 3 pS slots leaves a full exp of
             cross-iteration slack, so score matmuls rarely wait. The
             remaining 13 V row-tiles (paced ahead of their PV consumers)
             and the half-0 out-projection are drained into the PE stream
             as filler so exp waits never idle the PE; filler accumulates
             in a single spare PSUM bank.
  exp split: ACT (real exp) and DVE (one-op Schraudolph bf16:
             i16 = rint(s*128/ln2 + B) bitcast bf16, mean-zero calibrated,
             ~40% of chunks; softmax renormalizes, output err ~0.5%).
  norm:      po evacuated to bf16 SBUF immediately (frees PSUM banks);
             denominator row round-trips through DRAM reshaped to [128, 8]
             so the reciprocal uses 128 DVE lanes (0.13us vs 6.5us for a
             [64,1024] broadcast reciprocal); stride-0 DMA broadcast; one
             2x-mode bf16 multiply into oT_sb.
  epilogue:  half-1 out-projection, PSUM->SBUF copies alternating ACT/DVE.

b_v is folded into the V matmul ones-row bias; out_b added on host.
"""

import sys

sys.path.insert(0, "/opt/trn_rl_repo")

import math
from contextlib import ExitStack

import ml_dtypes
import numpy as np

import concourse.bass as bass
import concourse.tile as tile
from concourse import mybir
from concourse.alu_op_type import AluOpType
from concourse.bass_utils import run_bass_kernel_spmd

BF16 = ml_dtypes.bfloat16
F32 = mybir.dt.float32
BF = mybir.dt.bfloat16
I16 = mybir.dt.int16

T = 2048
D = 1024
H = 16
HD = 64
R = 16
BSZ = 2
SCALE = 16.0
NCORES = 8
HPC = 4  # heads per core
CD = HPC * HD  # 256 head dims per core
VW = HD + 1  # V block width per head (ones column appended)
KPAD = 1152  # 1024 X rows + 1 ones row, padded to 9 k-tiles of 128
NKT = KPAD // 128
P = 128
NTT = T // P  # 16 row tiles
HF = T // 2  # 1024: ti processed in two halves

# Schraudolph-bf16 exp: i16 = rint(x * 128/ln2 + (127*128 - C)), bitcast bf16
EXP_A = 128.0 / math.log(2.0)
EXP_B = 127.0 * 128.0 - 7.3
# tj tiles whose c==1 exp chunk goes to DVE-Schraudolph (rest go to ACT)
DVE_TJ = frozenset(range(16)) - {5, 10, 15}


def build_nc():
    nc = bass.Bass()
    xa = nc.dram_tensor("xa", [KPAD, T], BF, kind="ExternalInput")
    wqk = nc.dram_tensor("wqk", [KPAD, 2 * CD], BF, kind="ExternalInput")
    wv = nc.dram_tensor("wv", [KPAD, HPC * VW], BF, kind="ExternalInput")
    ab = nc.dram_tensor("ab", [KPAD, 3 * R], BF, kind="ExternalInput")
    kbm = nc.dram_tensor("kbm", [R, CD], BF, kind="ExternalInput")
    vbm = nc.dram_tensor("vbm", [R, HPC * VW], BF, kind="ExternalInput")
    wo = nc.dram_tensor("wo", [CD, D], BF, kind="ExternalInput")
    out = nc.dram_tensor("out", [T, D], F32, kind="ExternalOutput")

    with tile.TileContext(nc) as tc, ExitStack() as ctx:
        singles = ctx.enter_context(tc.tile_pool(name="singles", bufs=1))

        xa_t = [singles.tile([P, T], BF, name=f"xa{i}", tag=f"xa{i}") for i in range(NKT)]
        wqk_t = [singles.tile([P, 2 * CD], BF, name=f"wqk{i}", tag=f"wqk{i}") for i in range(NKT)]
        wv_t = [singles.tile([P, HPC * VW], BF, name=f"wv{i}", tag=f"wv{i}") for i in range(NKT)]
        ab_t = [singles.tile([P, 3 * R], BF, name=f"ab{i}", tag=f"ab{i}") for i in range(NKT)]
        kb_t = singles.tile([P, CD], BF, tag="kb")
        vb_t = singles.tile([P, HPC * VW], BF, tag="vb")
        nc.vector.memset(kb_t, 0.0)
        nc.vector.memset(vb_t, 0.0)
        wo_t = [singles.tile([P, D], BF, name=f"wo{i}", tag=f"wo{i}") for i in range(2)]
        # load order matches consumption: A needs ab+xa, then B needs wqk
        for i in range(NKT):
            nc.sync.dma_start(out=ab_t[i], in_=ab[i * P : (i + 1) * P, :])
            nc.sync.dma_start(out=xa_t[i], in_=xa[i * P : (i + 1) * P, :])
        for i in range(NKT):
            nc.sync.dma_start(out=wqk_t[i], in_=wqk[i * P : (i + 1) * P, :])
        for i in range(NKT):
            nc.sync.dma_start(out=wv_t[i], in_=wv[i * P : (i + 1) * P, :])
        nc.sync.dma_start(out=kb_t[0:R, :], in_=kbm[:, :])
        nc.sync.dma_start(out=vb_t[0:R, :], in_=vbm[:, :])
        for i in range(2):
            nc.sync.dma_start(out=wo_t[i], in_=wo[i * P : (i + 1) * P, :])

        # Q^T tiles (heads 0-1 / 2-3); K^T stored per head zero-padded to
        # 128 contraction rows so every attention matmul runs in the PE's
        # (128,128) tiling mode -- mode switches drain the whole array.
        qk_sb = [singles.tile([P, T], BF, name=f"qk{i}", tag=f"qk{i}") for i in range(2)]
        kp_sb = [singles.tile([P, T], BF, name=f"kp{i}", tag=f"kp{i}") for i in range(HPC)]
        ak_sb = singles.tile([P, T], BF, tag="ak")
        av_sb = singles.tile([P, T], BF, tag="av")
        for t8 in kp_sb:
            nc.vector.memset(t8, 0.0)
        nc.vector.memset(ak_sb, 0.0)
        nc.vector.memset(av_sb, 0.0)
        v_sb = [singles.tile([P, HPC * VW], BF, name=f"v{i}", tag=f"v{i}") for i in range(NTT)]
        oT_sb = [singles.tile([P, T], BF, name=f"oT{i}", tag=f"oT{i}") for i in range(2)]
        pOut = ctx.enter_context(tc.tile_pool(name="pOut", bufs=3))
        ob_sb = {}

        def emit_b_chunk(pool, m, ch, copy_engine):
            cs = slice(ch * 512, (ch + 1) * 512)
            pq = pool.tile([P, 512], F32, tag="aux", name=f"pq_{m}_{ch}")
            steps = []
            for kt in range(NKT):
                steps.append(
                    lambda kt=kt, pq=pq: nc.tensor.matmul(
                        pq,
                        lhsT=wqk_t[kt][:, m * P : (m + 1) * P],
                        rhs=xa_t[kt][:, cs],
                        start=(kt == 0),
                        stop=(kt == NKT - 1 and m < 2),
                    )
                )
            if m >= 2:
                steps.append(
                    lambda pq=pq: nc.tensor.matmul(
                        pq,
                        lhsT=kb_t[:, (m - 2) * P : (m - 1) * P],
                        rhs=ak_sb[:, cs],
                        start=False,
                        stop=True,
                    )
                )
                h0 = 2 * (m - 2)
                steps.append(
                    lambda pq=pq, h0=h0: copy_engine(
                        kp_sb[h0][0:HD, cs], pq[0:HD, :]
                    )
                )
                steps.append(
                    lambda pq=pq, h0=h0: copy_engine(
                        kp_sb[h0 + 1][HD:P, cs], pq[HD:P, :]
                    )
                )
            else:
                steps.append(lambda pq=pq: copy_engine(qk_sb[m][:, cs], pq))
            return steps

        def emit_c_chunk(pool, mt, copy_engine):
            ms = slice(mt * P, (mt + 1) * P)
            pv = pool.tile([P, HPC * VW], F32, tag="aux", name=f"pv_{mt}")
            steps = []
            for kt in range(NKT):
                steps.append(
                    lambda kt=kt, pv=pv: nc.tensor.matmul(
                        pv,
                        lhsT=xa_t[kt][:, ms],
                        rhs=wv_t[kt],
                        start=(kt == 0),
                        stop=False,
                    )
                )
            steps.append(
                lambda pv=pv: nc.tensor.matmul(
                    pv, lhsT=av_sb[:, ms], rhs=vb_t, start=False, stop=True
                )
            )
            steps.append(lambda pv=pv: copy_engine(v_sb[mt], pv))
            return steps

        def emit_outproj_chunk(pool, mt, ch, copy_engine):
            ms = slice(mt * P, (mt + 1) * P)
            cs = slice(ch * 512, (ch + 1) * 512)
            po2 = pool.tile([P, 512], F32, tag="aux", name=f"po2_{mt}_{ch}")
            steps = []
            if ch == 0:
                def mkob(mt=mt):
                    ob_sb[mt] = pOut.tile([P, D], F32, tag="ob", name=f"ob_{mt}")
                steps.append(mkob)
            for kt2 in range(2):
                steps.append(
                    lambda kt2=kt2, po2=po2: nc.tensor.matmul(
                        po2,
                        lhsT=oT_sb[kt2][:, ms],
                        rhs=wo_t[kt2][:, cs],
                        start=(kt2 == 0),
                        stop=(kt2 == 1),
                    )
                )

            def fin(po2=po2, mt=mt, ch=ch):
                copy_engine(ob_sb[mt][:, cs], po2)
                if ch == 1:
                    nc.sync.dma_start(out=out[ms, :], in_=ob_sb[mt])

            steps.append(fin)
            return steps

        # Phase A: A_kv^T = [k_a; v_a] @ X   (48, T); kt-outer so each
        # matmul consumes an xa k-tile as soon as its DMA lands
        with tc.tile_pool(name="pA", bufs=4, space="PSUM") as pA:
            pa4 = [pA.tile([3 * R, 512], F32, tag="pa", name=f"pa{ch}") for ch in range(4)]
            for kt in range(8):  # ab rows >= 1024 are zero; skip 9th tile
                for ch in range(4):
                    nc.tensor.matmul(
                        pa4[ch],
                        lhsT=ab_t[kt],
                        rhs=xa_t[kt][:, ch * 512 : (ch + 1) * 512],
                        start=(kt == 0),
                        stop=(kt == 7),
                    )
            for ch in range(4):
                cs = slice(ch * 512, (ch + 1) * 512)
                nc.vector.tensor_copy(ak_sb[0:R, cs], pa4[ch][0:R, :])
                nc.vector.tensor_copy(av_sb[0:R, cs], pa4[ch][2 * R : 3 * R, :])

        # Prologue: just enough of Q^T/K^T for heads 0-1 (m0 half-0 cols,
        # all of m2) plus V row-tiles 0-2; the rest of B and C drains into
        # the attention stream as filler.
        with tc.tile_pool(name="pPro", bufs=3, space="PSUM") as pPro:
            for m, ch in [(2, 0), (2, 1), (2, 2), (2, 3), (0, 0), (0, 1)]:
                for step in emit_b_chunk(pPro, m, ch, nc.vector.tensor_copy):
                    step()
            for mt in range(3):
                for step in emit_c_chunk(pPro, mt, nc.vector.tensor_copy):
                    step()

        # Attention: 8 single-head units with interleaved filler
        with (
            tc.tile_pool(name="pS", bufs=4, space="PSUM") as pS,
            tc.tile_pool(name="pO", bufs=3, space="PSUM") as pO,
            tc.tile_pool(name="pX", bufs=1, space="PSUM") as pX,
            tc.tile_pool(name="pP", bufs=6) as pP,
            tc.tile_pool(name="pEv", bufs=3) as pEv,
            tc.tile_pool(name="pN", bufs=3) as pN,
            tc.tile_pool(name="pD", bufs=3, space="DRAM") as pD,
        ):
            filler = []

            def alt_copy(i=[0]):
                i[0] += 1
                return nc.vector.tensor_copy if i[0] % 2 else nc.scalar.copy

            # queue order respects consumer deadlines: C[mt] before unit
            # (0,0)'s PV(mt); m3+m1(half0) before unit (0,2); m0(half1)
            # before unit (1,0); m1(half1) before unit (1,2)
            for mt in range(3, NTT):
                filler.extend(emit_c_chunk(pX, mt, alt_copy()))
            for m, ch in [(3, 0), (3, 1), (3, 2), (3, 3), (1, 0), (1, 1),
                          (0, 2), (0, 3), (1, 2), (1, 3)]:
                filler.extend(emit_b_chunk(pX, m, ch, alt_copy()))
            fill_pos = [0]

            def drain_filler(n):
                i = fill_pos[0]
                for _ in range(n):
                    if i >= len(filler):
                        break
                    filler[i]()
                    i += 1
                fill_pos[0] = i

            def emit_unit(half, h, fill_rate):
                hp = h // 2
                po = [
                    pO.tile([VW, 512], F32, tag="po", name=f"po_{half}_{h}_{c}")
                    for c in range(2)
                ]
                pts = {}

                def emit_pv(t):
                    pt = pts.pop(t)
                    for c in range(2):
                        nc.tensor.matmul(
                            po[c],
                            lhsT=v_sb[t][:, h * VW : (h + 1) * VW],
                            rhs=pt[:, c * 512 : (c + 1) * 512],
                            start=(t == 0),
                            stop=(t == NTT - 1),
                        )

                for tj in range(NTT):
                    pt = pP.tile([P, HF], BF, tag="pt", name=f"pt_{half}_{h}_{tj}")
                    for c in range(2):
                        s_ps = pS.tile([P, 512], F32, tag="s", name=f"ps_{half}_{h}_{tj}_{c}")
                        nc.tensor.matmul(
                            s_ps,
                            lhsT=kp_sb[h][:, tj * P : (tj + 1) * P],
                            rhs=qk_sb[hp][:, half * HF + c * 512 : half * HF + (c + 1) * 512],
                            start=True,
                            stop=True,
                        )
                        ptc = pt[:, c * 512 : (c + 1) * 512]
                        if c == 1 and tj in DVE_TJ:
                            nc.vector.tensor_scalar(
                                ptc.bitcast(I16), s_ps, EXP_A, EXP_B,
                                AluOpType.mult, AluOpType.add,
                            )
                        else:
                            nc.scalar.activation(
                                ptc, s_ps, mybir.ActivationFunctionType.Exp
                            )
                    pts[tj] = pt
                    drain_filler(fill_rate)
                    if tj > 1:
                        emit_pv(tj - 2)
                emit_pv(NTT - 2)
                emit_pv(NTT - 1)
                return po

            def emit_norm(half, h, po):
                hs = slice(half * HF, (half + 1) * HF)
                ev = pEv.tile([VW, HF], BF, tag="ev", name=f"ev_{half}_{h}")
                for c in range(2):
                    nc.vector.tensor_copy(ev[:, c * 512 : (c + 1) * 512], po[c])
                dr = pD.tile([1, HF], BF, tag="dr", name=f"dr_{half}_{h}")
                nc.sync.dma_start(out=dr, in_=ev[HD:VW, :])
                den128 = pN.tile([P, HF // P], BF, tag="d128", name=f"d128_{half}_{h}")
                nc.sync.dma_start(
                    out=den128,
                    in_=bass.AP(tensor=dr.tensor, offset=dr.offset,
                                ap=[[HF // P, P], [1, HF // P]]),
                )
                rec = pN.tile([P, HF // P], BF, tag="rec", name=f"rec_{half}_{h}")
                with nc.allow_low_precision(
                    reason="softmax denom ~2048; bf16 recip adds ~0.4% row scale noise"
                ):
                    nc.vector.reciprocal(rec, den128)
                rr = pD.tile([1, HF], BF, tag="rr", name=f"rr_{half}_{h}")
                nc.sync.dma_start(
                    out=bass.AP(tensor=rr.tensor, offset=rr.offset,
                                ap=[[HF // P, P], [1, HF // P]]),
                    in_=rec,
                )
                rb = pN.tile([HD, HF], BF, tag="rb", name=f"rb_{half}_{h}")
                nc.sync.dma_start(
                    out=rb,
                    in_=bass.AP(tensor=rr.tensor, offset=rr.offset,
                                ap=[[0, HD], [1, HF]]),
                )
                nc.vector.tensor_mul(
                    oT_sb[h // 2][(h % 2) * HD : (h % 2) * HD + HD, hs],
                    ev[0:HD, :],
                    rb,
                )

            prev = None
            for half in range(2):
                for h in range(HPC):
                    rate = {(0, 0): 9, (0, 1): 6, (0, 2): 3}.get((half, h), 2)
                    po = emit_unit(half, h, rate)
                    if prev is not None:
                        emit_norm(*prev)
                        if (half, h) == (1, 0):
                            for mt in range(NTT // 2):
                                for ch in range(2):
                                    filler.extend(
                                        emit_outproj_chunk(pX, mt, ch, alt_copy())
                                    )
                    prev = (half, h, po)
            emit_norm(*prev)
            drain_filler(len(filler))

        # Epilogue: half-1 out-projection
        with tc.tile_pool(name="pE", bufs=2, space="PSUM") as pE:
            eng = [nc.vector.tensor_copy, nc.scalar.copy]
            for mt in range(NTT // 2, NTT):
                for ch in range(2):
                    for step in emit_outproj_chunk(pE, mt, ch, eng[(mt + ch) % 2]):
                        step()

    import bass_rust as _bass_rust

    _bass_rust.move_matmul_waits_to_ldweights(nc.m)
    _bass_rust.generate_event_semaphores(nc)
    return nc


def prepare_in_maps(inputs):
    q = np.asarray(inputs["query"], np.float32)
    ipw = np.asarray(inputs["in_proj_weight"], np.float32)
    ipb = np.asarray(inputs["in_proj_bias"], np.float32)
    out_w = np.asarray(inputs["out_w"], np.float32)
    k_a = np.asarray(inputs["k_a"], np.float32)
    k_b = np.asarray(inputs["k_b"], np.float32)
    v_a = np.asarray(inputs["v_a"], np.float32)
    v_b = np.asarray(inputs["v_b"], np.float32)
    qscale = 1.0 / math.sqrt(HD)
    sl = SCALE / R

    in_maps = []
    for c in range(NCORES):
        bb = c // 4
        s = (c % 4) * CD
        e = s + CD
        X = q[:, bb, :]

        xa = np.zeros((KPAD, T), np.float32)
        xa[:D] = X.T
        xa[D] = 1.0

        wqk = np.zeros((KPAD, 2 * CD), np.float32)
        wqk[:D, :CD] = ipw[s:e].T * qscale
        wqk[D, :CD] = ipb[s:e] * qscale
        wqk[:D, CD:] = ipw[D + s : D + e].T
        wqk[D, CD:] = ipb[D + s : D + e]

        wv = np.zeros((KPAD, HPC * VW), np.float32)
        for j in range(HPC):
            wv[:D, j * VW : j * VW + HD] = ipw[2 * D + s + j * HD : 2 * D + s + (j + 1) * HD].T
            wv[D, j * VW : j * VW + HD] = ipb[2 * D + s + j * HD : 2 * D + s + (j + 1) * HD]
            wv[D, j * VW + HD] = 1.0

        ab = np.zeros((KPAD, 3 * R), np.float32)
        ab[:D, :R] = k_a.T
        ab[:D, 2 * R :] = v_a.T

        kbm = k_b[:, s:e] * sl

        vbm = np.zeros((R, HPC * VW), np.float32)
        for j in range(HPC):
            vbm[:, j * VW : j * VW + HD] = v_b[:, s + j * HD : s + (j + 1) * HD] * sl

        wo = out_w[:, s:e].T

        in_maps.append(
            {
                "xa": xa.astype(BF16),
                "wqk": wqk.astype(BF16),
                "wv": wv.astype(BF16),
                "ab": ab.astype(BF16),
                "kbm": kbm.astype(BF16),
                "vbm": vbm.astype(BF16),
                "wo": wo.astype(BF16),
            }
        )
    return in_maps


def assemble_output(inputs, results):
    out_b = np.asarray(inputs["out_b"], np.float32)
    out = np.zeros((T, BSZ, D), np.float32)
    for c in range(NCORES):
        out[:, c // 4, :] += results[c]["out"]
    out += out_b[None, None, :]
    return out


def kernel(**inputs):
    nc = build_nc()
    in_maps = prepare_in_maps(inputs)
    res = run_bass_kernel_spmd(nc, in_maps, core_ids=list(range(NCORES)))
    return assemble_output(inputs, res.results)
